# revision 41
# baseline (speedup 1.0000x reference)
"""Bass/Tile Trainium2 kernel for nn_Attention_37538014167301.

GQA attention layer (B=2, S=2048, E=2048, H=16, KVH=4, D=128) with RoPE and
causal softmax, sharded over 8 NeuronCores: batch (2-way) x head-group
(4-way tensor parallel).  Core c handles batch b=c//4 and q heads
{g, g+4, g+8, g+12} with g=c%4; under torch-style .repeat (jnp.tile) GQA,
those q heads all use kv head g, so each core needs exactly one kv head.

Everything on device is kept in transposed [dim, seq] layout so that every
matmul contracts over the partition axis:
  - projections compute Q^T/K^T/V^T = W @ x^T directly (bf16 inputs,
    fp32 PSUM accumulation),
  - RoPE is applied in [d, s] layout using a pair-swap permutation matmul
    plus elementwise ops against host-precomputed cos/sin tables,
  - scores are computed transposed (P^T[sk, sq]) in fp32r so that P^T can
    feed the attention*V matmul (bf16) with V in natural [sk, d] layout,
  - the softmax denominator is accumulated on DVE in bf16 (2x mode) and
    reduced across partitions with one ones-matmul per (pair, chunk),
  - the output projection (bf16) accumulates y^T[f, s] per core; the host
    sums the four head-group partials per batch.

v3 (emit_attention_v3) splits PSUM into three independent rings
(projections | scores | attention accumulators) so the Tile scheduler can
overlap next-chunk projections and the previous chunk's output projection
with the exp-latency-bound attention loop.  The timing loop (reps>1) uses
For_i with staggered semaphore reset + all-engine branch hints.
"""

import math

import numpy as np

B, S, E = 2, 2048, 2048
H, KVH = 16, 4
D = E // H  # 128
N_CORES = 8
GROUPS = 4  # head groups (tensor-parallel degree per batch)
HQ = H // GROUPS  # q heads per core


# ---------------------------------------------------------------------------
# Device program
# ---------------------------------------------------------------------------

def emit_attention(tc, io, S_=S, E_=E, HQ_=HQ, D_=D, CH=512, XSPLIT=4):
    """Emit the per-core attention program into TileContext tc.

    io: dict of DRAM APs: xT, wqT, wkT, wvT, woT, cosd, sind, swp, yT
    """
    import concourse.mybir as mybir
    from concourse.masks import make_identity

    nc = tc.nc
    f32 = mybir.dt.float32
    f32r = mybir.dt.float32r

    NE = E_ // 128        # contraction tiles over e
    XSPLIT = min(XSPLIT, NE)
    NSQ = S_ // CH        # q chunks
    R = CH // 128         # sk tiles per q chunk width
    NSK = S_ // 128       # sk tiles
    DQ = HQ_ * D_
    scale = 1.0 / math.sqrt(D_)

    xT, wqT, wkT, wvT, woT = io["xT"], io["wqT"], io["wkT"], io["wvT"], io["woT"]
    cosd, sind, swp, yT = io["cosd"], io["sind"], io["swp"], io["yT"]

    from contextlib import ExitStack

    with ExitStack() as ctx:
        const = ctx.enter_context(tc.tile_pool(name="const", bufs=1))
        persist = ctx.enter_context(tc.tile_pool(name="persist", bufs=1))
        pacc = ctx.enter_context(tc.tile_pool(name="pacc", bufs=2, space="PSUM"))
        pden = ctx.enter_context(tc.tile_pool(name="pden", bufs=2, space="PSUM"))

        cos_sb = const.tile([D_, S_], f32, name="cos_sb")
        nc.sync.dma_start(out=cos_sb, in_=cosd)
        sin_sb = const.tile([D_, S_], f32, name="sin_sb")
        nc.sync.dma_start(out=sin_sb, in_=sind)
        swp_sb = const.tile([D_, D_], f32r, name="swp_sb")
        nc.sync.dma_start(out=swp_sb, in_=swp)
        ones_f = const.tile([128, 1], f32, name="ones_f")
        nc.gpsimd.memset(ones_f, 1.0)
        ones_sb = const.tile([128, 1], f32r, name="ones_sb")
        nc.vector.tensor_copy(out=ones_sb, in_=ones_f)
        ident_sb = const.tile([128, 128], f32, name="ident_sb")
        make_identity(nc, ident_sb)

        q_rot = [persist.tile([D_, S_], f32r, name=f"qrot{m}") for m in range(HQ_)]
        k_rot = persist.tile([D_, S_], f32r, name="k_rot")
        v_nat = persist.tile([128, NSK, D_], f32r, name="v_nat")

        # ------------------------------------------------------------------
        # Phase A: projections + rope, chunk by chunk over s
        # ------------------------------------------------------------------
        with ExitStack() as actx:
            wpool = actx.enter_context(tc.tile_pool(name="wpool", bufs=1))
            xpool = actx.enter_context(tc.tile_pool(name="xpool", bufs=2 * XSPLIT))
            work = actx.enter_context(tc.tile_pool(name="awork", bufs=3))
            pmm = actx.enter_context(tc.tile_pool(name="pmm", bufs=4, space="PSUM"))

            # weights arrive as [E, n] = [(t p), n] -> [p, t, n] sbuf layout.
            # Small K/V weights first so the first projection group's inputs
            # (wk + x chunk 0) aren't stuck behind the 4MB wq transfer.
            wk_sb = wpool.tile([128, NE, D_], f32r, name="wk_sb")
            nc.sync.dma_start(out=wk_sb,
                              in_=wkT.rearrange("(t p) n -> p t n", p=128))
            wv_sb = wpool.tile([128, NE, D_], f32r, name="wv_sb")
            nc.sync.dma_start(out=wv_sb,
                              in_=wvT.rearrange("(t p) n -> p t n", p=128))

            xT_r = xT.rearrange("(t p) s -> p t s", p=128)  # [128, NE, S]
            TG = NE // XSPLIT  # e-tiles per x DMA
            wq_sb = wpool.tile([128, NE, DQ], f32r, name="wq_sb")
            wqT_r = wqT.rearrange("(t p) n -> p t n", p=128)

            for j in range(NSQ):
                ssl = slice(CH * j, CH * (j + 1))
                x_chunks = []
                for u in range(XSPLIT):
                    xc = xpool.tile([128, TG, CH], f32r, name="xc", tag="xc")
                    nc.sync.dma_start(
                        out=xc, in_=xT_r[:, TG * u:TG * (u + 1), ssl])
                    x_chunks.append(xc)
                x_tiles = [x_chunks[t // TG][:, t % TG, :] for t in range(NE)]
                if j == 0:
                    # wq arrives after the first x chunk, in 4 e-tile groups
                    for u in range(XSPLIT):
                        nc.sync.dma_start(
                            out=wq_sb[:, TG * u:TG * (u + 1), :],
                            in_=wqT_r[:, TG * u:TG * (u + 1), :])

                def project(w_slices, n):
                    ps = pmm.tile([128, CH], f32, name="ps_proj", tag="mm")
                    for t in range(NE):
                        nc.tensor.matmul(
                            ps[:n, :], w_slices(t), x_tiles[t],
                            start=(t == 0), stop=(t == NE - 1),
                        )
                    return ps

                def rope(ps, dst):
                    # dst[:, ssl] = ps * cos + (SWAP @ ps) * sin_signed
                    p_sb = work.tile([D_, CH], f32r, name="rope_in", tag="rope_in")
                    nc.vector.tensor_copy(out=p_sb, in_=ps[:D_, :])
                    ps2 = pmm.tile([128, CH], f32, name="ps_swap", tag="mm")
                    nc.tensor.matmul(ps2[:D_, :], swp_sb, p_sb,
                                     start=True, stop=True)
                    t1 = work.tile([D_, CH], f32, name="rope_t1", tag="rope_t1")
                    nc.vector.tensor_mul(t1, p_sb.bitcast(f32), cos_sb[:, ssl])
                    t2 = work.tile([D_, CH], f32, name="rope_t2", tag="rope_t2")
                    nc.vector.tensor_mul(t2, ps2[:D_, :], sin_sb[:, ssl])
                    nc.vector.tensor_add(dst[:, ssl], t1, t2)

                # K
                ps = project(lambda t: wk_sb[:, t, :], D_)
                rope(ps, k_rot)
                # V: copy to sbuf, then PE-transpose each 128x128 block
                ps = project(lambda t: wv_sb[:, t, :], D_)
                vt_sb = work.tile([D_, CH], f32, name="vt_sb", tag="vt_sb")
                nc.vector.tensor_copy(out=vt_sb, in_=ps[:D_, :])
                for u in range(R):
                    pvt = pmm.tile([128, CH], f32, name="ps_vt", tag="mm")
                    nc.tensor.transpose(pvt[:, :D_], vt_sb[:, 128 * u:128 * (u + 1)],
                                        ident_sb)
                    nc.vector.tensor_copy(out=v_nat[:, R * j + u, :], in_=pvt[:, :D_])
                # Q heads
                for m in range(HQ_):
                    ps = project(lambda t: wq_sb[:, t, D_ * m:D_ * (m + 1)], D_)
                    rope(ps, q_rot[m])

        # ------------------------------------------------------------------
        # Phase B: attention per (head, q chunk); Phase C: output projection
        # ------------------------------------------------------------------
        with ExitStack() as bctx:
            bpool = bctx.enter_context(tc.tile_pool(name="bpool", bufs=1))
            pwork = bctx.enter_context(tc.tile_pool(name="pwork", bufs=4))
            nwork = bctx.enter_context(tc.tile_pool(name="nwork", bufs=2))
            psc_pool = bctx.enter_context(
                tc.tile_pool(name="psc", bufs=2, space="PSUM"))

            o_sb = [bpool.tile([D_, S_], f32r, name=f"osb{m}") for m in range(HQ_)]

            wopool = bctx.enter_context(tc.tile_pool(name="wopool", bufs=1))
            ywork = bctx.enter_context(tc.tile_pool(name="ywork", bufs=3))
            wo_sb = wopool.tile([128, HQ_, E_], f32r, name="wo_sb")
            for m in range(HQ_):
                nc.sync.dma_start(out=wo_sb[:, m, :], in_=woT[128 * m:128 * (m + 1), :])

            assert HQ_ % 2 == 0
            for j in range(NSQ):
                ssl = slice(CH * j, CH * (j + 1))
                for hp in range(HQ_ // 2):
                    m0, m1 = 2 * hp, 2 * hp + 1
                    n_i = R * (j + 1)
                    po0 = pacc.tile([128, CH], f32, name="po0", tag="acc")
                    po1 = pacc.tile([128, CH], f32, name="po1", tag="acc")
                    pd0 = pden.tile([1, CH], f32, name="pd0", tag="den")
                    pd1 = pden.tile([1, CH], f32, name="pd1", tag="den")
                    for i in range(n_i):
                        # diagonal tiles: columns < 128*u are fully below the
                        # causal boundary -> skip them in scores/exp/PV/den;
                        # only the 128-wide strip [128u, 128(u+1)) needs a mask
                        diag = i >= R * j
                        u = i - R * j if diag else 0
                        lo = 128 * u
                        csl = slice(lo, CH)
                        qsl = slice(CH * j + lo, CH * (j + 1))
                        # transposed scores for both heads into one 2-bank psum
                        psc = psc_pool.tile([128, 2, CH], f32, name="psc",
                                            tag="sc")
                        nc.tensor.matmul(
                            psc[:, 0, csl], k_rot[:, 128 * i:128 * (i + 1)],
                            q_rot[m0][:, qsl], start=True, stop=True)
                        nc.tensor.matmul(
                            psc[:, 1, csl], k_rot[:, 128 * i:128 * (i + 1)],
                            q_rot[m1][:, qsl], start=True, stop=True)
                        p_sb = pwork.tile([128, 2, CH], f32r, name="p_sb",
                                          tag="p_sb")
                        nc.scalar.activation(out=p_sb[:, :, csl],
                                             in_=psc[:, :, csl],
                                             func=mybir.ActivationFunctionType.Exp,
                                             scale=scale)
                        if diag:
                            # keep where sk <= sq, i.e. strip col c' >= p
                            nc.gpsimd.affine_select(
                                out=p_sb[:, :, lo:lo + 128],
                                in_=p_sb[:, :, lo:lo + 128],
                                compare_op=mybir.AluOpType.is_ge,
                                fill=0.0,
                                base=0,
                                pattern=[[0, 2], [1, 128]],
                                channel_multiplier=-1,
                            )
                        last = i == n_i - 1
                        nc.tensor.matmul(po0[:, csl], v_nat[:, i, :],
                                         p_sb[:, 0, csl],
                                         start=(i == 0), stop=last)
                        nc.tensor.matmul(po1[:, csl], v_nat[:, i, :],
                                         p_sb[:, 1, csl],
                                         start=(i == 0), stop=last)
                        nc.tensor.matmul(pd0[:, csl], ones_sb,
                                         p_sb[:, 0, csl],
                                         start=(i == 0), stop=last)
                        nc.tensor.matmul(pd1[:, csl], ones_sb,
                                         p_sb[:, 1, csl],
                                         start=(i == 0), stop=last)
                    for mm, po, pd in ((m0, po0, pd0), (m1, po1, pd1)):
                        # copy unnormalized out to sbuf on ACT so the psum
                        # accumulator frees without waiting for the
                        # recip/broadcast chain
                        ou = nwork.tile([D_, CH], f32, name="ou", tag="ou")
                        nc.scalar.activation(
                            out=ou, in_=po[:D_, :],
                            func=mybir.ActivationFunctionType.Copy)
                        recip = nwork.tile([1, CH], f32, name="recip",
                                           tag="recip")
                        nc.vector.reciprocal(out=recip, in_=pd)
                        rbc = nwork.tile([128, CH], f32, name="rbc", tag="rbc")
                        nc.gpsimd.partition_broadcast(rbc, recip)
                        nc.vector.tensor_mul(o_sb[mm][:, ssl], ou,
                                             rbc[:D_, :])

                # output projection for this s-chunk (pipelines behind
                # attention of the next chunk)
                for tf in range(NE):
                    fsl = slice(128 * tf, 128 * (tf + 1))
                    py = pacc.tile([128, CH], f32, name="py", tag="acc")
                    for m in range(HQ_):
                        nc.tensor.matmul(py, wo_sb[:, m, fsl],
                                         o_sb[m][:, ssl],
                                         start=(m == 0), stop=(m == HQ_ - 1))
                    y_sb = ywork.tile([128, CH], f32, name="y_sb", tag="y_sb")
                    nc.any.tensor_copy(out=y_sb, in_=py)
                    nc.sync.dma_start(out=yT[fsl, ssl], in_=y_sb)


def emit_attention_v2(tc, io, S_=S, E_=E, HQ_=HQ, D_=D, CH=512, XSPLIT=8,
                      phases="ABC"):
    """Fused emission: per s-chunk, projections + rope + attention are
    interleaved so PE has dense work across what were phase boundaries.
    Output projection stays a final phase (SBUF can't hold wo alongside the
    projection working set).

    PSUM budget (8 banks): psc pool 2 bufs x [128,2,CH] (4) + pacc 2 + pden 2.
    Projection accumulators, rope-swap outputs and V-transposes share psc
    slots in head pairs.
    """
    import concourse.mybir as mybir
    from concourse.masks import make_identity

    nc = tc.nc
    f32 = mybir.dt.float32
    f32r = mybir.dt.float32r

    NE = E_ // 128
    XSPLIT = min(XSPLIT, NE)
    NSQ = S_ // CH
    R = CH // 128
    NSK = S_ // 128
    DQ = HQ_ * D_
    NP = HQ_ // 2  # head pairs
    scale = 1.0 / math.sqrt(D_)
    assert HQ_ % 2 == 0

    xT, wqT, wkT, wvT, woT = io["xT"], io["wqT"], io["wkT"], io["wvT"], io["woT"]
    cosd, sind, swp, yT = io["cosd"], io["sind"], io["swp"], io["yT"]

    from contextlib import ExitStack

    with ExitStack() as ctx:
        const = ctx.enter_context(tc.tile_pool(name="const", bufs=1))
        persist = ctx.enter_context(tc.tile_pool(name="persist", bufs=1))
        qpool = ctx.enter_context(tc.tile_pool(name="qpool", bufs=2))
        cpool = ctx.enter_context(tc.tile_pool(name="cpool", bufs=2))
        pacc = ctx.enter_context(tc.tile_pool(name="pacc", bufs=2, space="PSUM"))
        pden = ctx.enter_context(tc.tile_pool(name="pden", bufs=2, space="PSUM"))
        psc_pool = ctx.enter_context(
            tc.tile_pool(name="psc", bufs=2, space="PSUM"))

        swp_sb = const.tile([D_, D_], f32r, name="swp_sb")
        nc.sync.dma_start(out=swp_sb, in_=swp)
        ones_f = const.tile([128, 1], f32, name="ones_f")
        nc.gpsimd.memset(ones_f, 1.0)
        ones_sb = const.tile([128, 1], f32r, name="ones_sb")
        nc.vector.tensor_copy(out=ones_sb, in_=ones_f)
        ident_sb = const.tile([128, 128], f32, name="ident_sb")
        make_identity(nc, ident_sb)

        k_rot = persist.tile([D_, S_], f32r, name="k_rot")
        v_nat = persist.tile([128, NSK, D_], f32r, name="v_nat")
        o_sb = [persist.tile([D_, S_], f32r, name=f"osb{m}")
                for m in range(HQ_)]

        with ExitStack() as actx:
            wpool = actx.enter_context(tc.tile_pool(name="wpool", bufs=1))
            xpool = actx.enter_context(tc.tile_pool(name="xpool", bufs=XSPLIT))
            work = actx.enter_context(tc.tile_pool(name="awork", bufs=2))
            pwork = actx.enter_context(tc.tile_pool(name="pwork", bufs=3))
            nwork = actx.enter_context(tc.tile_pool(name="nwork", bufs=2))

            wk_sb = wpool.tile([128, NE, D_], f32r, name="wk_sb")
            nc.sync.dma_start(out=wk_sb,
                              in_=wkT.rearrange("(t p) n -> p t n", p=128))
            wv_sb = wpool.tile([128, NE, D_], f32r, name="wv_sb")
            nc.sync.dma_start(out=wv_sb,
                              in_=wvT.rearrange("(t p) n -> p t n", p=128))

            xT_r = xT.rearrange("(t p) s -> p t s", p=128)
            TG = NE // XSPLIT
            wq_sb = wpool.tile([128, NE, DQ], f32r, name="wq_sb")
            wqT_r = wqT.rearrange("(t p) n -> p t n", p=128)

            for j in range(NSQ):
                ssl = slice(CH * j, CH * (j + 1))
                x_chunks = []
                for u in range(XSPLIT):
                    xc = xpool.tile([128, TG, CH], f32r, name="xc", tag="xc")
                    nc.sync.dma_start(
                        out=xc, in_=xT_r[:, TG * u:TG * (u + 1), ssl])
                    x_chunks.append(xc)
                x_tiles = [x_chunks[t // TG][:, t % TG, :] for t in range(NE)]
                if j == 0:
                    for u in range(XSPLIT):
                        nc.sync.dma_start(
                            out=wq_sb[:, TG * u:TG * (u + 1), :],
                            in_=wqT_r[:, TG * u:TG * (u + 1), :])

                cos_c = cpool.tile([D_, CH], f32, name="cos_c", tag="cos_c")
                nc.sync.dma_start(out=cos_c, in_=cosd[:, ssl])
                sin_c = cpool.tile([D_, CH], f32, name="sin_c", tag="sin_c")
                nc.sync.dma_start(out=sin_c, in_=sind[:, ssl])

                # --- K+V projections into one paired psum slot ---
                pkv = psc_pool.tile([128, 2, CH], f32, name="pkv", tag="sc")
                for t in range(NE):
                    nc.tensor.matmul(pkv[:, 0, :], wk_sb[:, t, :], x_tiles[t],
                                     start=(t == 0), stop=(t == NE - 1))
                for t in range(NE):
                    nc.tensor.matmul(pkv[:, 1, :], wv_sb[:, t, :], x_tiles[t],
                                     start=(t == 0), stop=(t == NE - 1))

                # --- K rope + V transpose reuse the pkv slot banks ---
                rink = work.tile([D_, 2, CH], f32r, name="rin", tag="rin")
                nc.vector.tensor_copy(out=rink[:, 0, :], in_=pkv[:, 0, :])
                nc.tensor.matmul(pkv[:, 0, :], swp_sb, rink[:, 0, :],
                                 start=True, stop=True)
                t1k = work.tile([D_, 2, CH], f32, name="t1", tag="t1")
                nc.vector.tensor_mul(t1k[:, 0, :], rink[:, 0, :].bitcast(f32),
                                     cos_c)
                t2k = work.tile([D_, 2, CH], f32, name="t2", tag="t2")
                nc.vector.tensor_mul(t2k[:, 0, :], pkv[:, 0, :], sin_c)
                nc.vector.tensor_add(k_rot[:, ssl], t1k[:, 0, :], t2k[:, 0, :])

                vt_sb = work.tile([D_, CH], f32, name="vt_sb", tag="vt_sb")
                nc.vector.tensor_copy(out=vt_sb, in_=pkv[:, 1, :])
                for u in range(R):
                    nc.tensor.transpose(pkv[:, 1, 128 * u:128 * (u + 1)],
                                        vt_sb[:, 128 * u:128 * (u + 1)],
                                        ident_sb)
                    nc.vector.tensor_copy(out=v_nat[:, R * j + u, :],
                                          in_=pkv[:, 1, 128 * u:128 * (u + 1)])

                # --- Q projections + rope, in head pairs ---
                qp = []
                for p in range(NP):
                    m0 = 2 * p
                    pq = psc_pool.tile([128, 2, CH], f32, name="pq", tag="sc")
                    for h in range(2):
                        wsl = slice(D_ * (m0 + h), D_ * (m0 + h + 1))
                        for t in range(NE):
                            nc.tensor.matmul(
                                pq[:, h, :], wq_sb[:, t, wsl], x_tiles[t],
                                start=(t == 0), stop=(t == NE - 1))
                    rin = work.tile([D_, 2, CH], f32r, name="rin", tag="rin")
                    nc.vector.tensor_copy(out=rin, in_=pq[:D_, :, :])
                    for h in range(2):
                        nc.tensor.matmul(pq[:D_, h, :], swp_sb, rin[:, h, :],
                                         start=True, stop=True)
                    cos_b = cos_c[:, None, :].broadcast_to([D_, 2, CH])
                    sin_b = sin_c[:, None, :].broadcast_to([D_, 2, CH])
                    t1 = work.tile([D_, 2, CH], f32, name="t1", tag="t1")
                    nc.vector.tensor_mul(t1, rin.bitcast(f32), cos_b)
                    t2 = work.tile([D_, 2, CH], f32, name="t2", tag="t2")
                    nc.vector.tensor_mul(t2, pq[:D_, :, :], sin_b)
                    qrot = qpool.tile([D_, 2, CH], f32r, name=f"qrot{p}",
                                      tag=f"qrot{p}")
                    nc.vector.tensor_add(qrot, t1, t2)
                    qp.append(qrot)

                # --- attention for this chunk ---
                for p in (range(NP) if "B" in phases else ()):
                    m0, m1 = 2 * p, 2 * p + 1
                    n_i = R * (j + 1)
                    po0 = pacc.tile([128, CH], f32, name="po0", tag="acc")
                    po1 = pacc.tile([128, CH], f32, name="po1", tag="acc")
                    pd0 = pden.tile([1, CH], f32, name="pd0", tag="den")
                    pd1 = pden.tile([1, CH], f32, name="pd1", tag="den")
                    for i in range(n_i):
                        diag = i >= R * j
                        u = i - R * j if diag else 0
                        lo = 128 * u
                        csl = slice(lo, CH)
                        W = CH - lo
                        # merge the head pair into one matmul when the
                        # combined moving size fits the 512 fp32 limit
                        merged = False  # CoreSim can't validate strided pair matmuls
                        psc = psc_pool.tile([128, 2, CH], f32, name="psc",
                                            tag="sc")
                        ksl = k_rot[:, 128 * i:128 * (i + 1)]
                        if merged:
                            nc.tensor.matmul(psc[:, :, csl], ksl,
                                             qp[p][:, :, csl],
                                             start=True, stop=True)
                        else:
                            nc.tensor.matmul(psc[:, 0, csl], ksl,
                                             qp[p][:, 0, csl],
                                             start=True, stop=True)
                            nc.tensor.matmul(psc[:, 1, csl], ksl,
                                             qp[p][:, 1, csl],
                                             start=True, stop=True)
                        p_sb = pwork.tile([128, 2, CH], f32r, name="p_sb",
                                          tag="p_sb")
                        nc.scalar.activation(
                            out=p_sb[:, :, csl], in_=psc[:, :, csl],
                            func=mybir.ActivationFunctionType.Exp, scale=scale)
                        if diag:
                            nc.gpsimd.affine_select(
                                out=p_sb[:, :, lo:lo + 128],
                                in_=p_sb[:, :, lo:lo + 128],
                                compare_op=mybir.AluOpType.is_ge,
                                fill=0.0, base=0,
                                pattern=[[0, 2], [1, 128]],
                                channel_multiplier=-1,
                            )
                        last = i == n_i - 1
                        if merged:
                            nc.tensor.matmul(po0[:, csl], v_nat[:, i, :],
                                             p_sb[:, 0, csl],
                                             start=(i == 0), stop=last)
                            nc.tensor.matmul(pd0[:, csl], ones_sb,
                                             p_sb[:, 0, csl],
                                             start=(i == 0), stop=last)
                            nc.tensor.matmul(pd1[:, csl], ones_sb,
                                             p_sb[:, 1, csl],
                                             start=(i == 0), stop=last)
                        else:
                            # stop=True closes each matmul's psum group so the
                            # paired po tile never has two pending groups;
                            # has_written persists, so accumulation continues
                            nc.tensor.matmul(po0[:, csl], v_nat[:, i, :],
                                             p_sb[:, 0, csl],
                                             start=(i == 0), stop=last)
                            nc.tensor.matmul(po1[:, csl], v_nat[:, i, :],
                                             p_sb[:, 1, csl],
                                             start=(i == 0), stop=last)
                            nc.tensor.matmul(pd0[:, csl], ones_sb,
                                             p_sb[:, 0, csl],
                                             start=(i == 0), stop=last)
                            nc.tensor.matmul(pd1[:, csl], ones_sb,
                                             p_sb[:, 1, csl],
                                             start=(i == 0), stop=last)
                    for mm, po, pd in ((m0, po0, pd0), (m1, po1, pd1)):
                        ou = nwork.tile([D_, CH], f32, name="ou", tag="ou")
                        nc.scalar.activation(
                            out=ou, in_=po[:D_, :],
                            func=mybir.ActivationFunctionType.Copy)
                        recip = nwork.tile([1, CH], f32, name="recip",
                                           tag="recip")
                        nc.vector.reciprocal(out=recip, in_=pd)
                        rbc = nwork.tile([128, CH], f32, name="rbc", tag="rbc")
                        nc.gpsimd.partition_broadcast(rbc, recip)
                        nc.vector.tensor_mul(o_sb[mm][:, ssl], ou,
                                             rbc[:D_, :])

        # --- output projection (phase C) ---
        if "C" not in phases:
            return
        with ExitStack() as cctx:
            wopool = cctx.enter_context(tc.tile_pool(name="wopool", bufs=1))
            ywork = cctx.enter_context(tc.tile_pool(name="ywork", bufs=3))
            wo_sb = wopool.tile([128, HQ_, E_], f32r, name="wo_sb")
            for m in range(HQ_):
                nc.sync.dma_start(out=wo_sb[:, m, :],
                                  in_=woT[128 * m:128 * (m + 1), :])
            for tf in range(NE):
                fsl = slice(128 * tf, 128 * (tf + 1))
                for j in range(NSQ):
                    ssl = slice(CH * j, CH * (j + 1))
                    py = pacc.tile([128, CH], f32, name="py", tag="acc")
                    for m in range(HQ_):
                        nc.tensor.matmul(py, wo_sb[:, m, fsl],
                                         o_sb[m][:, ssl],
                                         start=(m == 0), stop=(m == HQ_ - 1))
                    y_sb = ywork.tile([128, CH], f32, name="y_sb", tag="y_sb")
                    nc.any.tensor_copy(out=y_sb, in_=py)
                    nc.sync.dma_start(out=yT[fsl, ssl], in_=y_sb)


def emit_attention_v3(tc, io, S_=S, E_=E, HQ_=HQ, D_=D, CH=512, XSPLIT=8,
                      merge_strip=True):
    """v3: scheduler-friendly restructure of v2.

    - PSUM split into three independent rings so the Tile scheduler can run
      projections (next chunk), attention (current chunk) and the output
      projection (current chunk) concurrently:
        pproj 2x[128,CH] (2 banks) | psc 2x[128,2,CH] (4) | pacc 2x[128,CH] (2)
    - softmax denominator accumulated on DVE in bf16 (2x mode) instead of
      per-tile PE matmuls; a single ones-matmul per (pair, chunk) does the
      final 128-partition reduction.
    - p_sb / V / o / wo / y in bf16: halves DVE+DMA cost, PE rate unchanged.
    - rope PSUM->SBUF copies on ACT, cos-muls on GpSimd, sin-muls + final
      adds on DVE (spreads the elementwise load off DVE).
    - output projection emitted per chunk so it fills PE gaps left by the
      exp-latency-bound attention loop.
    """
    import concourse.mybir as mybir
    from concourse.masks import make_identity

    nc = tc.nc
    f32 = mybir.dt.float32
    f32r = mybir.dt.float32r
    bf16 = mybir.dt.bfloat16

    NE = E_ // 128
    XSPLIT = min(XSPLIT, NE)
    NSQ = S_ // CH
    R = CH // 128
    NSK = S_ // 128
    DQ = HQ_ * D_
    NP = HQ_ // 2
    scale = 1.0 / math.sqrt(D_)
    assert HQ_ % 2 == 0

    xT, wqT, wkT, wvT, woT = io["xT"], io["wqT"], io["wkT"], io["wvT"], io["woT"]
    cosd, sind, swp, yT = io["cosd"], io["sind"], io["swp"], io["yT"]

    from contextlib import ExitStack

    with ExitStack() as ctx:
        const = ctx.enter_context(tc.tile_pool(name="const", bufs=1))
        persist = ctx.enter_context(tc.tile_pool(name="persist", bufs=1))
        wpool = ctx.enter_context(tc.tile_pool(name="wpool", bufs=2))
        xpool = ctx.enter_context(tc.tile_pool(name="xpool", bufs=2 * XSPLIT))
        cpool = ctx.enter_context(tc.tile_pool(name="cpool", bufs=2))
        qpool = ctx.enter_context(tc.tile_pool(name="qpool", bufs=2))
        rwork = ctx.enter_context(tc.tile_pool(name="rwork", bufs=2))
        pwork = ctx.enter_context(tc.tile_pool(name="pwork", bufs=4))
        dpool = ctx.enter_context(tc.tile_pool(name="dpool", bufs=2))
        nwork = ctx.enter_context(tc.tile_pool(name="nwork", bufs=2))
        ywork = ctx.enter_context(tc.tile_pool(name="ywork", bufs=6))
        pproj = ctx.enter_context(tc.tile_pool(name="pproj", bufs=2,
                                               space="PSUM"))
        psc_pool = ctx.enter_context(tc.tile_pool(name="psc", bufs=2,
                                                  space="PSUM"))
        pacc = ctx.enter_context(tc.tile_pool(name="pacc", bufs=2,
                                              space="PSUM"))

        swp_sb = const.tile([D_, D_], f32r, name="swp_sb")
        nc.sync.dma_start(out=swp_sb, in_=swp)
        ones_f = const.tile([128, 1], f32, name="ones_f")
        nc.gpsimd.memset(ones_f, 1.0)
        ones_bf = const.tile([128, 1], bf16, name="ones_bf")
        nc.vector.tensor_copy(out=ones_bf, in_=ones_f)
        ident_f = const.tile([128, 128], f32, name="ident_f")
        make_identity(nc, ident_f)
        ident_bf = const.tile([128, 128], bf16, name="ident_bf")
        nc.vector.tensor_copy(out=ident_bf, in_=ident_f)

        k_rot = persist.tile([D_, S_], f32r, name="k_rot")
        v_nat = persist.tile([128, NSK, D_], bf16, name="v_nat")
        o_sb = [persist.tile([D_, S_], bf16, name=f"osb{m}")
                for m in range(HQ_)]

        # weight tiles; DMA emission is ordered inside chunk 0 so the serial
        # DMA stream matches first-use order: cos/sin, wk, x, wv, wq, wo
        wk_sb = wpool.tile([128, NE, D_], bf16, name="wk_sb")
        wv_sb = wpool.tile([128, NE, D_], bf16, name="wv_sb")
        wo_sb = wpool.tile([128, HQ_, E_], bf16, name="wo_sb")

        xT_r = xT.rearrange("(t p) s -> p t s", p=128)
        TG = NE // XSPLIT
        wq_sb = wpool.tile([128, NE, DQ], bf16, name="wq_sb")
        wqT_r = wqT.rearrange("(t p) n -> p t n", p=128)

        for j in range(NSQ):
            ssl = slice(CH * j, CH * (j + 1))
            cos_c = cpool.tile([D_, CH], f32, name="cos_c", tag="cos_c")
            nc.sync.dma_start(out=cos_c, in_=cosd[:, ssl])
            sin_c = cpool.tile([D_, CH], f32, name="sin_c", tag="sin_c")
            nc.sync.dma_start(out=sin_c, in_=sind[:, ssl])
            if j == 0:
                nc.sync.dma_start(out=wk_sb,
                                  in_=wkT.rearrange("(t p) n -> p t n", p=128))
            x_chunks = []
            for u in range(XSPLIT):
                xc = xpool.tile([128, TG, CH], bf16, name="xc", tag="xc")
                nc.sync.dma_start(out=xc, in_=xT_r[:, TG * u:TG * (u + 1), ssl])
                x_chunks.append(xc)
            x_tiles = [x_chunks[t // TG][:, t % TG, :] for t in range(NE)]
            if j == 0:
                nc.sync.dma_start(out=wv_sb,
                                  in_=wvT.rearrange("(t p) n -> p t n", p=128))
                for u in range(XSPLIT):
                    nc.sync.dma_start(
                        out=wq_sb[:, TG * u:TG * (u + 1), :],
                        in_=wqT_r[:, TG * u:TG * (u + 1), :])
                for m in range(HQ_):
                    nc.sync.dma_start(out=wo_sb[:, m, :],
                                      in_=woT[128 * m:128 * (m + 1), :])

            # ---- K projection + rope ----
            pk = pproj.tile([128, CH], f32, name="pk", tag="proj")
            for t in range(NE):
                nc.tensor.matmul(pk[:D_, :], wk_sb[:, t, :], x_tiles[t],
                                 start=(t == 0), stop=(t == NE - 1))
            krin = rwork.tile([D_, CH], f32r, name="krin", tag="krin", bufs=1)
            nc.scalar.activation(out=krin, in_=pk[:D_, :],
                                 func=mybir.ActivationFunctionType.Copy)
            nc.tensor.matmul(pk[:D_, :], swp_sb, krin, start=True, stop=True)
            kt1 = rwork.tile([D_, CH], f32, name="kt1", tag="kt1", bufs=1)
            nc.gpsimd.tensor_mul(kt1, krin.bitcast(f32), cos_c)
            kt2 = rwork.tile([D_, CH], f32, name="kt2", tag="kt2", bufs=1)
            nc.vector.tensor_mul(kt2, pk[:D_, :], sin_c)
            nc.vector.tensor_add(k_rot[:, ssl], kt1, kt2)

            # ---- V projection + transpose ----
            pv = pproj.tile([128, CH], f32, name="pv", tag="proj")
            for t in range(NE):
                nc.tensor.matmul(pv[:D_, :], wv_sb[:, t, :], x_tiles[t],
                                 start=(t == 0), stop=(t == NE - 1))
            vt_sb = rwork.tile([D_, CH], bf16, name="vt_sb", tag="vt_sb")
            nc.scalar.activation(out=vt_sb, in_=pv[:D_, :],
                                 func=mybir.ActivationFunctionType.Copy)
            pvb = pv.bitcast(bf16)  # reuse the bank as bf16 transpose scratch
            for u in range(R):
                nc.tensor.transpose(pvb[:, 128 * u:128 * (u + 1)],
                                    vt_sb[:, 128 * u:128 * (u + 1)], ident_bf)
                nc.vector.tensor_copy(out=v_nat[:, R * j + u, :],
                                      in_=pvb[:, 128 * u:128 * (u + 1)])

            # ---- Q projections + rope, in head pairs ----
            qp = []
            for p in range(NP):
                rin = rwork.tile([D_, 2, CH], f32r, name="rin", tag="rin")
                qt2 = rwork.tile([D_, 2, CH], f32, name="qt2", tag="qt2")
                for h in range(2):
                    m = 2 * p + h
                    pq = pproj.tile([128, CH], f32, name="pq", tag="proj")
                    wsl = slice(D_ * m, D_ * (m + 1))
                    for t in range(NE):
                        nc.tensor.matmul(pq[:D_, :], wq_sb[:, t, wsl],
                                         x_tiles[t],
                                         start=(t == 0), stop=(t == NE - 1))
                    nc.scalar.activation(
                        out=rin[:, h, :], in_=pq[:D_, :],
                        func=mybir.ActivationFunctionType.Copy)
                    nc.tensor.matmul(pq[:D_, :], swp_sb, rin[:, h, :],
                                     start=True, stop=True)
                    nc.vector.tensor_mul(qt2[:, h, :], pq[:D_, :], sin_c)
                cos_b = cos_c[:, None, :].broadcast_to([D_, 2, CH])
                qt1 = rwork.tile([D_, 2, CH], f32, name="qt1", tag="qt1")
                nc.gpsimd.tensor_mul(qt1, rin.bitcast(f32), cos_b)
                qrot = qpool.tile([D_, 2, CH], f32r, name=f"qrot{p}",
                                  tag=f"qrot{p}")
                nc.vector.tensor_add(qrot, qt1, qt2)
                qp.append(qrot)

            # ---- attention, per head pair ----
            for p in range(NP):
                m0, m1 = 2 * p, 2 * p + 1
                n_i = R * (j + 1)
                po0 = pacc.tile([128, CH], f32, name="po0", tag="acc")
                po1 = pacc.tile([128, CH], f32, name="po1", tag="acc")
                dacc = dpool.tile([128, 2, CH], bf16, name="dacc", tag="dacc")
                for i in range(n_i):
                    diag = i >= R * j
                    u = i - R * j if diag else 0
                    lo = 128 * u
                    csl = slice(lo, CH)
                    psc = psc_pool.tile([128, 2, CH], f32, name="psc",
                                        tag="sc")
                    ksl = k_rot[:, 128 * i:128 * (i + 1)]
                    p_sb = pwork.tile([128, 2, CH], bf16, name="p_sb",
                                      tag="p_sb")
                    if merge_strip and diag and CH - lo == 128:
                        # 128-wide strip: merge the head pair into one matmul
                        # (moving [D,2,128] packed h-major into one bank) to
                        # dodge the fp32r free-dim<256 rate penalty
                        nc.tensor.matmul(psc[:, 0, 0:256], ksl,
                                         qp[p][:, :, csl],
                                         start=True, stop=True)
                        nc.scalar.activation(
                            out=p_sb[:, :, csl], in_=psc[:, 0, 0:256],
                            func=mybir.ActivationFunctionType.Exp,
                            scale=scale)
                    else:
                        nc.tensor.matmul(psc[:, 0, csl], ksl,
                                         qp[p][:, 0, csl],
                                         start=True, stop=True)
                        nc.tensor.matmul(psc[:, 1, csl], ksl,
                                         qp[p][:, 1, csl],
                                         start=True, stop=True)
                        nc.scalar.activation(
                            out=p_sb[:, :, csl], in_=psc[:, :, csl],
                            func=mybir.ActivationFunctionType.Exp,
                            scale=scale)
                    if diag:
                        nc.gpsimd.affine_select(
                            out=p_sb[:, :, lo:lo + 128],
                            in_=p_sb[:, :, lo:lo + 128],
                            compare_op=mybir.AluOpType.is_ge,
                            fill=0.0, base=0,
                            pattern=[[0, 2], [1, 128]],
                            channel_multiplier=-1,
                        )
                    if i == 0:
                        nc.vector.tensor_copy(out=dacc, in_=p_sb)
                    else:
                        nc.vector.tensor_add(dacc[:, :, csl], dacc[:, :, csl],
                                             p_sb[:, :, csl])
                    last = i == n_i - 1
                    nc.tensor.matmul(po0[:, csl], v_nat[:, i, :],
                                     p_sb[:, 0, csl],
                                     start=(i == 0), stop=last)
                    nc.tensor.matmul(po1[:, csl], v_nat[:, i, :],
                                     p_sb[:, 1, csl],
                                     start=(i == 0), stop=last)
                # final denominator reduction + normalization (per head so
                # the recip->broadcast->mul chain pipelines)
                den_t = psc_pool.tile([1, 2, CH], f32, name="den_t", tag="sc")
                for h, (mm, po) in enumerate(((m0, po0), (m1, po1))):
                    nc.tensor.matmul(den_t[:, h, :], ones_bf, dacc[:, h, :],
                                     start=True, stop=True)
                    recip = nwork.tile([1, CH], f32, name="recip",
                                       tag="recip", bufs=2)
                    nc.vector.reciprocal(out=recip, in_=den_t[:, h, :])
                    rbc = nwork.tile([128, CH], f32, name="rbc", tag="rbc")
                    nc.gpsimd.partition_broadcast(rbc, recip)
                    nc.vector.tensor_mul(o_sb[mm][:, ssl], po[:D_, :],
                                         rbc[:D_, :])

            # ---- output projection, one chunk late: emitted after the NEXT
            # chunk's attention in program order, it has higher scheduler
            # priority index and so fills PE stalls left by the exp-latency-
            # bound attention loop of chunk j (which has no proj filler on
            # the last chunk).
            def emit_outproj(jc):
                osl = slice(CH * jc, CH * (jc + 1))
                for tf in range(NE):
                    fsl = slice(128 * tf, 128 * (tf + 1))
                    py = pproj.tile([128, CH], f32, name="py", tag="proj")
                    for m in range(HQ_):
                        nc.tensor.matmul(py, wo_sb[:, m, fsl],
                                         o_sb[m][:, osl],
                                         start=(m == 0), stop=(m == HQ_ - 1))
                    y_sb = ywork.tile([128, CH], bf16, name="y_sb",
                                      tag="y_sb")
                    if tf % 2 == 0:
                        nc.vector.tensor_copy(out=y_sb, in_=py)
                    else:
                        nc.scalar.activation(
                            out=y_sb, in_=py,
                            func=mybir.ActivationFunctionType.Copy)
                    nc.sync.dma_start(out=yT[fsl, osl], in_=y_sb)

            if j > 0:
                emit_outproj(j - 1)
        emit_outproj(NSQ - 1)


def emit_attention_v4(tc, io, S_=S, E_=E, HQ_=HQ, D_=D, CH=512, XSPLIT=4,
                      mid_cb=None):
    """v4: all-bf16 matmul datapath + cheap normalization + no GpSimd libs.

    Changes vs v3 (trace-driven):
    - k_rot / q_rot / rope internals in bf16: the f32r score+swap matmuls
      measured ~2.4x slower than bf16 on HW (84.7us for 172 matmuls).
    - softmax denominator reciprocal via Ln->Exp(-x) on ACT (both funcs in
      the natural_log_exp_and_others table, so no ACT table reloads) and a
      ones-matmul partition broadcast on PE; replaces the [1,CH] DVE
      reciprocal (3.3us each, 53us total) + GpSimd partition_broadcast
      (whose custom-ucode library swaps cost ~4us each).
    - causal diagonal mask via a constant [128,128] triangular bf16 mask
      multiplied on DVE; replaces gpsimd.affine_select.  With
      partition_broadcast also gone, GpSimd runs zero custom libraries.
    - unnormalized attention output copied PSUM->SBUF on DVE right after
      the last PV matmul so the PSUM accumulator frees without waiting
      for the denominator chain.
    - chunk-0 DMA order: wk, x, cos/sin, wv, wq, wo (first-use order).
    - y written back one DMA per 4 f-tiles.
    """
    import concourse.mybir as mybir
    from concourse.masks import make_identity

    nc = tc.nc
    f32 = mybir.dt.float32
    f32r = mybir.dt.float32r
    bf16 = mybir.dt.bfloat16

    NE = E_ // 128
    XSPLIT = min(XSPLIT, NE)
    NSQ = S_ // CH
    R = CH // 128
    NSK = S_ // 128
    DQ = HQ_ * D_
    NP = HQ_ // 2
    scale = 1.0 / math.sqrt(D_)
    assert HQ_ % 2 == 0

    xT, wqT, wkT, wvT, woT = io["xT"], io["wqT"], io["wkT"], io["wvT"], io["woT"]
    cosd, sind, swp, tri, yT = (io["cosd"], io["sind"], io["swp"], io["tri"],
                                io["yT"])

    from contextlib import ExitStack

    with ExitStack() as ctx:
        const = ctx.enter_context(tc.tile_pool(name="const", bufs=1))
        persist = ctx.enter_context(tc.tile_pool(name="persist", bufs=1))
        wpool = ctx.enter_context(tc.tile_pool(name="wpool", bufs=2))
        xpool = ctx.enter_context(tc.tile_pool(name="xpool", bufs=2))
        cpool = ctx.enter_context(tc.tile_pool(name="cpool", bufs=2))
        qpool = ctx.enter_context(tc.tile_pool(name="qpool", bufs=2))
        rwork = ctx.enter_context(tc.tile_pool(name="rwork", bufs=2))
        pwork = ctx.enter_context(tc.tile_pool(name="pwork", bufs=4))
        dpool = ctx.enter_context(tc.tile_pool(name="dpool", bufs=3))
        nwork = ctx.enter_context(tc.tile_pool(name="nwork", bufs=2))
        ywork = ctx.enter_context(tc.tile_pool(name="ywork", bufs=3))
        pproj = ctx.enter_context(tc.tile_pool(name="pproj", bufs=2,
                                               space="PSUM"))
        psc_pool = ctx.enter_context(tc.tile_pool(name="psc", bufs=2,
                                                  space="PSUM"))
        pacc = ctx.enter_context(tc.tile_pool(name="pacc", bufs=2,
                                              space="PSUM"))

        swp_sb = const.tile([D_, D_], bf16, name="swp_sb")
        tri_sb = const.tile([128, 128], bf16, name="tri_sb")
        ones_f = const.tile([128, 1], f32, name="ones_f")
        nc.gpsimd.memset(ones_f, 1.0)
        ones_bf = const.tile([128, 1], bf16, name="ones_bf")
        nc.vector.tensor_copy(out=ones_bf, in_=ones_f)
        onesr_f = const.tile([1, 128], f32, name="onesr_f")
        nc.gpsimd.memset(onesr_f, 1.0)
        onesr_bf = const.tile([1, 128], bf16, name="onesr_bf")
        nc.vector.tensor_copy(out=onesr_bf, in_=onesr_f)
        ident_f = const.tile([128, 128], f32, name="ident_f")
        make_identity(nc, ident_f)
        ident_bf = const.tile([128, 128], bf16, name="ident_bf")
        nc.vector.tensor_copy(out=ident_bf, in_=ident_f)

        k_rot = persist.tile([D_, S_], bf16, name="k_rot")
        v_nat = persist.tile([128, NSK, D_], bf16, name="v_nat")
        o_sb = [persist.tile([D_, S_], bf16, name=f"osb{m}")
                for m in range(HQ_)]

        wk_sb = wpool.tile([128, NE, D_], bf16, name="wk_sb")
        wv_sb = wpool.tile([128, NE, D_], bf16, name="wv_sb")
        wo_sb = wpool.tile([128, HQ_, E_], bf16, name="wo_sb")
        wq_sb = wpool.tile([128, NE, DQ], bf16, name="wq_sb")

        TG = NE // XSPLIT
        yT_r = yT.rearrange("(t p) s -> p t s", p=128)
        xT_r = xT.rearrange("(t p) s -> p t s", p=128)
        wkT_r = wkT.rearrange("(t p) n -> p t n", p=128)
        wqT_r = wqT.rearrange("(t p) n -> p t n", p=128)
        wvT_r = wvT.rearrange("(t p) n -> p t n", p=128)

        # x DMA split sizes: small first group so the first projection
        # matmul of a loop body can start ~2us after the DMA restart
        XSPL = [2, 7, 7] if NE == 16 else [NE // XSPLIT] * XSPLIT
        for j in range(NSQ):
            if j == NSQ // 2 and mid_cb is not None:
                # staggered-reset stage boundary at the body midpoint: the
                # NEXT unrolled body's DMA restart then shares a stage with
                # this body's tail instead of waiting on a stage edge
                mid_cb()
            ssl = slice(CH * j, CH * (j + 1))
            if j == 0:
                nc.sync.dma_start(out=wk_sb[:, 0:2, :], in_=wkT_r[:, 0:2, :])
            x_chunks, x_tiles = [], []
            t_base = 0
            for u, tg in enumerate(XSPL):
                xc = xpool.tile([128, tg, CH], bf16, name="xc", tag=f"xc{u}")
                nc.sync.dma_start(out=xc, in_=xT_r[:, t_base:t_base + tg, ssl])
                x_chunks.append(xc)
                x_tiles.extend(xc[:, i, :] for i in range(tg))
                t_base += tg
                if j == 0 and u == 0:
                    nc.sync.dma_start(out=wk_sb[:, 2:NE, :],
                                      in_=wkT_r[:, 2:NE, :])
            cos_c = cpool.tile([D_, CH], bf16, name="cos_c", tag="cos_c")
            nc.sync.dma_start(out=cos_c, in_=cosd[:, ssl])
            sin_c = cpool.tile([D_, CH], f32, name="sin_c", tag="sin_c")
            nc.sync.dma_start(out=sin_c, in_=sind[:, ssl])
            if j == 0:
                nc.sync.dma_start(out=swp_sb, in_=swp)
                nc.sync.dma_start(out=tri_sb, in_=tri)
                nc.sync.dma_start(out=wv_sb, in_=wvT_r)
                for u in range(XSPLIT):
                    nc.sync.dma_start(
                        out=wq_sb[:, TG * u:TG * (u + 1), :],
                        in_=wqT_r[:, TG * u:TG * (u + 1), :])
                for m in range(HQ_):
                    nc.sync.dma_start(out=wo_sb[:, m, :],
                                      in_=woT[128 * m:128 * (m + 1), :])

            # ---- K projection + rope ----
            pk = pproj.tile([128, CH], f32, name="pk", tag="proj")
            for t in range(NE):
                nc.tensor.matmul(pk[:D_, :], wk_sb[:, t, :], x_tiles[t],
                                 start=(t == 0), stop=(t == NE - 1))
            krin = rwork.tile([D_, CH], bf16, name="krin", tag="krin", bufs=1)
            nc.scalar.activation(out=krin, in_=pk[:D_, :],
                                 func=mybir.ActivationFunctionType.Copy)
            kt2 = rwork.tile([D_, CH], bf16, name="kt2", tag="kt2", bufs=1)
            nc.tensor.matmul(pk[:D_, :], swp_sb, krin, start=True, stop=True)
            kt1 = rwork.tile([D_, CH], bf16, name="kt1", tag="kt1", bufs=1)
            nc.gpsimd.tensor_mul(kt1, krin, cos_c)
            nc.vector.tensor_mul(kt2, pk[:D_, :], sin_c)
            nc.vector.tensor_add(k_rot[:, ssl], kt1, kt2)

            # ---- V projection + transpose ----
            pv = pproj.tile([128, CH], f32, name="pv", tag="proj")
            for t in range(NE):
                nc.tensor.matmul(pv[:D_, :], wv_sb[:, t, :], x_tiles[t],
                                 start=(t == 0), stop=(t == NE - 1))
            vt_sb = rwork.tile([D_, CH], bf16, name="vt_sb", tag="vt_sb")
            nc.scalar.activation(out=vt_sb, in_=pv[:D_, :],
                                 func=mybir.ActivationFunctionType.Copy)
            pvb = pv.bitcast(bf16)  # reuse the bank as bf16 transpose scratch
            for u in range(R):
                nc.tensor.transpose(pvb[:, 128 * u:128 * (u + 1)],
                                    vt_sb[:, 128 * u:128 * (u + 1)], ident_bf)
                nc.vector.tensor_copy(out=v_nat[:, R * j + u, :],
                                      in_=pvb[:, 128 * u:128 * (u + 1)])

            # ---- Q projections + rope, in head pairs ----
            qp = []
            for p in range(NP):
                rin = rwork.tile([D_, 2, CH], bf16, name="rin", tag="rin")
                qt2 = rwork.tile([D_, 2, CH], bf16, name="qt2", tag="qt2")
                for h in range(2):
                    m = 2 * p + h
                    pq = pproj.tile([128, CH], f32, name="pq", tag="proj")
                    wsl = slice(D_ * m, D_ * (m + 1))
                    for t in range(NE):
                        nc.tensor.matmul(pq[:D_, :], wq_sb[:, t, wsl],
                                         x_tiles[t],
                                         start=(t == 0), stop=(t == NE - 1))
                    nc.scalar.activation(
                        out=rin[:, h, :], in_=pq[:D_, :],
                        func=mybir.ActivationFunctionType.Copy)
                    nc.tensor.matmul(pq[:D_, :], swp_sb, rin[:, h, :],
                                     start=True, stop=True)
                    nc.vector.tensor_mul(qt2[:, h, :], pq[:D_, :], sin_c)
                cos_b = cos_c[:, None, :].broadcast_to([D_, 2, CH])
                qt1 = rwork.tile([D_, 2, CH], bf16, name="qt1", tag="qt1")
                nc.gpsimd.tensor_mul(qt1, rin, cos_b)
                qrot = qpool.tile([D_, 2, CH], bf16, name=f"qrot{p}",
                                  tag=f"qrot{p}")
                nc.vector.tensor_add(qrot, qt1, qt2)
                qp.append(qrot)

            # ---- attention, per head pair ----
            tri_bc = tri_sb[:, None, :].broadcast_to([128, 2, 128])
            pending = []
            for p in range(NP):
                m0, m1 = 2 * p, 2 * p + 1
                n_i = R * (j + 1)
                po0 = pacc.tile([128, CH], f32, name="po0", tag="acc")
                po1 = pacc.tile([128, CH], f32, name="po1", tag="acc")
                dacc = dpool.tile([128, 2, CH], bf16, name="dacc", tag="dacc")
                for i in range(n_i):
                    diag = i >= R * j
                    u = i - R * j if diag else 0
                    lo = 128 * u
                    csl = slice(lo, CH)
                    W = CH - lo
                    psc = psc_pool.tile([128, 2, CH], f32, name="psc",
                                        tag="sc")
                    ksl = k_rot[:, 128 * i:128 * (i + 1)]
                    p_sb = pwork.tile([128, 2, CH], bf16, name="p_sb",
                                      tag="p_sb")
                    if diag and 2 * W <= 512:
                        # merge the head pair into one matmul: moving
                        # [D,2,W] packed h-major into one psum bank
                        nc.tensor.matmul(psc[:, 0, 0:2 * W], ksl,
                                         qp[p][:, :, csl],
                                         start=True, stop=True)
                        nc.scalar.activation(
                            out=p_sb[:, :, csl], in_=psc[:, 0, 0:2 * W],
                            func=mybir.ActivationFunctionType.Exp,
                            scale=scale)
                    else:
                        nc.tensor.matmul(psc[:, 0, csl], ksl,
                                         qp[p][:, 0, csl],
                                         start=True, stop=True)
                        nc.tensor.matmul(psc[:, 1, csl], ksl,
                                         qp[p][:, 1, csl],
                                         start=True, stop=True)
                        nc.scalar.activation(
                            out=p_sb[:, :, csl], in_=psc[:, :, csl],
                            func=mybir.ActivationFunctionType.Exp,
                            scale=scale)
                    if diag:
                        # keep where sk <= sq: strip col c >= row r
                        nc.vector.tensor_mul(p_sb[:, :, lo:lo + 128],
                                             p_sb[:, :, lo:lo + 128],
                                             tri_bc)
                    if i == 0:
                        nc.vector.tensor_copy(out=dacc, in_=p_sb)
                    else:
                        nc.vector.tensor_add(dacc[:, :, csl], dacc[:, :, csl],
                                             p_sb[:, :, csl])
                    last = i == n_i - 1
                    nc.tensor.matmul(po0[:, csl], v_nat[:, i, :],
                                     p_sb[:, 0, csl],
                                     start=(i == 0), stop=last)
                    nc.tensor.matmul(po1[:, csl], v_nat[:, i, :],
                                     p_sb[:, 1, csl],
                                     start=(i == 0), stop=last)
                # copy unnormalized out to SBUF (into o_sb, normalized in
                # place later) so the PSUM accumulators free immediately
                nc.vector.tensor_copy(out=o_sb[m0][:, ssl], in_=po0[:D_, :])
                nc.vector.tensor_copy(out=o_sb[m1][:, ssl], in_=po1[:D_, :])
                pending.append((dacc, m0, m1))

            # ---- output projection, one chunk late (fills PE stalls of
            # the exp-latency-bound attention loop) ----
            def emit_outproj(jc):
                osl = slice(CH * jc, CH * (jc + 1))
                for tg in range(NE // 4):
                    y4 = ywork.tile([128, 4, CH], bf16, name="y4", tag="y4")
                    for tq in range(4):
                        tf = 4 * tg + tq
                        fsl = slice(128 * tf, 128 * (tf + 1))
                        py = pproj.tile([128, CH], f32, name="py", tag="proj")
                        for m in range(HQ_):
                            nc.tensor.matmul(py, wo_sb[:, m, fsl],
                                             o_sb[m][:, osl],
                                             start=(m == 0),
                                             stop=(m == HQ_ - 1))
                        if tq % 2 == 0:
                            nc.vector.tensor_copy(out=y4[:, tq, :], in_=py)
                        else:
                            nc.scalar.activation(
                                out=y4[:, tq, :], in_=py,
                                func=mybir.ActivationFunctionType.Copy)
                    nc.sync.dma_start(out=yT_r[:, 4 * tg:4 * tg + 4, osl],
                                      in_=y4)

            # ---- deferred normalization: den reduce + reciprocal +
            # partition-broadcast + in-place scale.  Emitted after the
            # outproj filler so the PE-stream never stalls on the
            # reciprocal chain; o_sb(j) is only read by outproj(j), one
            # chunk later.
            def emit_norm():
                for dacc_, m0_, m1_ in pending:
                    den_t = psc_pool.tile([1, 2, CH], f32, name="den_t",
                                          tag="sc")
                    for h in range(2):
                        nc.tensor.matmul(den_t[:, h, :], ones_bf,
                                         dacc_[:, h, :],
                                         start=True, stop=True)
                    recip = nwork.tile([1, 2, CH], f32, name="recip",
                                       tag="recip", bufs=2)
                    nc.vector.reciprocal_approx_fast(out=recip, in_=den_t)
                    recip_bf = nwork.tile([1, 2, CH], bf16, name="recip_bf",
                                          tag="recip_bf", bufs=2)
                    nc.vector.tensor_copy(out=recip_bf, in_=recip)
                    for h, mm in ((0, m0_), (1, m1_)):
                        # tag "sc" (not "proj"): borrowing a projection slot
                        # here would block the next chunk's K projection on
                        # this chunk's reciprocal chain
                        pbc = psc_pool.tile([128, CH], f32, name="pbc",
                                            tag="sc")
                        nc.tensor.matmul(pbc, onesr_bf, recip_bf[:, h, :],
                                         start=True, stop=True)
                        nc.vector.tensor_mul(o_sb[mm][:, ssl],
                                             o_sb[mm][:, ssl], pbc[:D_, :])

            if j > 0:
                emit_outproj(j - 1)
            emit_norm()
        emit_outproj(NSQ - 1)


def build_nc(S_=S, E_=E, HQ_=HQ, D_=D, CH=512, n_cores=N_CORES, reps=1,
             version=4, phases="ABC", merge_strip=True):
    """Build and compile the per-core Bass program (same program on all cores).

    reps > 1 wraps the whole body in a hardware For_i loop (timing harness
    use only: amortizes host dispatch overhead across reps executions).
    """
    import concourse.mybir as mybir
    import concourse.tile as tile
    from concourse import bacc

    f32 = mybir.dt.float32
    f32r = mybir.dt.float32r
    DQ = HQ_ * D_

    nc = bacc.Bacc("TRN2", target_bir_lowering=False, debug=False,
                   num_devices=n_cores)
    bf16 = mybir.dt.bfloat16
    wo_dt = bf16 if version >= 3 else f32r
    y_dt = bf16 if version >= 3 else f32
    in_dt = bf16 if version >= 3 else f32r
    cos_dt = bf16 if version >= 4 else f32
    swp_dt = bf16 if version >= 4 else f32r
    io = {
        "xT": nc.dram_tensor("xT", [E_, S_], in_dt, kind="ExternalInput").ap(),
        "wqT": nc.dram_tensor("wqT", [E_, DQ], in_dt, kind="ExternalInput").ap(),
        "wkT": nc.dram_tensor("wkT", [E_, D_], in_dt, kind="ExternalInput").ap(),
        "wvT": nc.dram_tensor("wvT", [E_, D_], in_dt, kind="ExternalInput").ap(),
        "woT": nc.dram_tensor("woT", [DQ, E_], wo_dt, kind="ExternalInput").ap(),
        "cosd": nc.dram_tensor("cosd", [D_, S_], cos_dt, kind="ExternalInput").ap(),
        "sind": nc.dram_tensor("sind", [D_, S_], f32, kind="ExternalInput").ap(),
        "swp": nc.dram_tensor("swp", [D_, D_], swp_dt, kind="ExternalInput").ap(),
        "yT": nc.dram_tensor("yT", [E_, S_], y_dt, kind="ExternalOutput").ap(),
    }
    if version >= 4:
        io["tri"] = nc.dram_tensor("tri", [128, 128], bf16,
                                   kind="ExternalInput").ap()
    def emit(tc, mid_cb=None):
        if version == 4:
            emit_attention_v4(tc, io, S_=S_, E_=E_, HQ_=HQ_, D_=D_, CH=CH,
                              mid_cb=mid_cb)
        elif version == 3:
            emit_attention_v3(tc, io, S_=S_, E_=E_, HQ_=HQ_, D_=D_, CH=CH,
                              merge_strip=merge_strip)
        elif version == 2:
            emit_attention_v2(tc, io, S_=S_, E_=E_, HQ_=HQ_, D_=D_, CH=CH,
                              phases=phases)
        else:
            emit_attention(tc, io, S_=S_, E_=E_, HQ_=HQ_, D_=D_, CH=CH)

    with tile.TileContext(nc) as tc:
        if reps == 1:
            emit(tc)
        elif reps < 0:
            # straight-line unroll (-reps bodies, no For_i back-edge);
            # used to isolate the hardware loop's per-iteration sync cost
            for _ in range(-reps):
                emit(tc)
        else:
            # 2x unroll inside the hardware loop: the interior body-body
            # boundary has no branch / sem-reset, so the Tile scheduler
            # pipelines across it; only every other boundary pays the
            # back-edge (IRAM refetch + DMA restart) cost.
            unroll = 1
            if version >= 4:
                for u in (4, 2):
                    if reps % u == 0:
                        unroll = u
                        break
            with tc.For_i(0, reps // unroll, 1,
                          hint_engines=(mybir.EngineType.PE,
                                        mybir.EngineType.DVE,
                                        mybir.EngineType.Activation,
                                        mybir.EngineType.Pool,
                                        mybir.EngineType.SP),
                          staggered_reset=True):
                for ui in range(unroll):
                    emit(tc)
    nc.compile()
    return nc


# ---------------------------------------------------------------------------
# Host-side sharding / gather
# ---------------------------------------------------------------------------

def round_fp32r(a):
    """Round fp32 array to fp32r (11-bit mantissa, RNE), keeping fp32 layout."""
    b = np.ascontiguousarray(a, dtype=np.float32).view(np.uint32)
    b = b + np.uint32(0x7FF) + ((b >> np.uint32(12)) & np.uint32(1))
    b &= np.uint32(0xFFFFF000)
    return b.view(np.float32)


def shard_inputs(x, wq_w, wk_w, wv_w, wo_w, freqs_cos, freqs_sin,
                 S_=S, E_=E, HQ_=HQ, D_=D, groups=GROUPS, n_batch=B,
                 version=4):
    """Build per-core input maps (list ordered core 0..n-1, c = b*groups + g)."""
    import ml_dtypes
    bf = ml_dtypes.bfloat16
    x = np.asarray(x, dtype=np.float32)
    wq_w = np.asarray(wq_w, dtype=np.float32)
    wk_w = np.asarray(wk_w, dtype=np.float32)
    wv_w = np.asarray(wv_w, dtype=np.float32)
    wo_w = np.asarray(wo_w, dtype=np.float32)
    fc = np.asarray(freqs_cos, dtype=np.float32)  # [S, D/2]
    fs = np.asarray(freqs_sin, dtype=np.float32)

    cosd = np.repeat(fc.T, 2, axis=0)  # [D, S]
    sind = np.repeat(fs.T, 2, axis=0)
    sign = np.where(np.arange(D_) % 2 == 0, -1.0, 1.0).astype(np.float32)
    sind = sind * sign[:, None]
    if version >= 4:
        cosd = cosd.astype(bf)
    swp = np.zeros((D_, D_), dtype=np.float32)
    idx = np.arange(0, D_, 2)
    swp[idx, idx + 1] = 1.0
    swp[idx + 1, idx] = 1.0
    if version >= 4:
        swp = swp.astype(bf)  # exact 0/1 values
    r = np.arange(128)
    tri = (r[None, :] >= r[:, None]).astype(bf)  # keep col >= row

    NE = E_ // 128
    CHS = 512
    NSQ = S_ // CHS

    def pmajor_w(wT):  # [E, n] -> [128, NE, n]
        return np.ascontiguousarray(
            wT.reshape(NE, 128, wT.shape[1]).transpose(1, 0, 2)).astype(bf)

    in_maps = []
    for c in range(n_batch * groups):
        b, g = divmod(c, groups)
        heads = [g + groups * mm for mm in range(HQ_)]
        wq_rows = np.concatenate([wq_w[h * D_:(h + 1) * D_] for h in heads])  # [DQ, E]
        wo_cols = np.concatenate([wo_w[:, h * D_:(h + 1) * D_] for h in heads],
                                 axis=1)  # [E, DQ]
        woT = np.ascontiguousarray(wo_cols.T).astype(bf)
        if version >= 4:
            im = {
                "xT": np.ascontiguousarray(x[b].T).astype(bf),
                "wqT": np.ascontiguousarray(wq_rows.T).astype(bf),
                "wkT": np.ascontiguousarray(
                    wk_w[g * D_:(g + 1) * D_].T).astype(bf),
                "wvT": np.ascontiguousarray(
                    wv_w[g * D_:(g + 1) * D_].T).astype(bf),
                "woT": woT,
                "cosd": cosd,
                "sind": sind,
                "swp": swp,
                "tri": tri,
            }
        else:
            im = {
                "xT": np.ascontiguousarray(x[b].T).astype(bf),
                "wqT": np.ascontiguousarray(wq_rows.T).astype(bf),
                "wkT": np.ascontiguousarray(
                    wk_w[g * D_:(g + 1) * D_].T).astype(bf),
                "wvT": np.ascontiguousarray(
                    wv_w[g * D_:(g + 1) * D_].T).astype(bf),
                "woT": woT,
                "cosd": cosd,
                "sind": sind,
                "swp": swp,
            }
        in_maps.append(im)
    return in_maps


def gather_output(results, S_=S, E_=E, groups=GROUPS, n_batch=B):
    """results: list of per-core dicts with 'yT' [E, S] -> y [B, S, E]."""
    out = np.zeros((n_batch, S_, E_), dtype=np.float32)
    for c, res in enumerate(results):
        b = c // groups
        out[b] += res["yT"].T.astype(np.float32)
    return out


# ---------------------------------------------------------------------------
# Per-core numpy reference (for validation in tests)
# ---------------------------------------------------------------------------

def percore_ref(xT, wqT, wkT, wvT, woT, fc, fs, HQ_=HQ, D_=D):
    x = xT.T.astype(np.float64)  # [S, E]
    S_ = x.shape[0]
    q = x @ wqT.astype(np.float64)  # [S, DQ]
    k = x @ wkT.astype(np.float64)  # [S, D]
    v = x @ wvT.astype(np.float64)

    def rope(t):  # [S, n, D]
        tr = t.reshape(*t.shape[:-1], -1, 2)
        xr, xi = tr[..., 0], tr[..., 1]
        c = fc[:, None, :]
        s = fs[:, None, :]
        orr = xr * c - xi * s
        oi = xr * s + xi * c
        return np.stack([orr, oi], axis=-1).reshape(t.shape)

    q = rope(q.reshape(S_, HQ_, D_))
    k = rope(k.reshape(S_, 1, D_))[:, 0]
    out = np.zeros((S_, HQ_ * D_))
    causal = np.tril(np.ones((S_, S_), dtype=bool))
    for m in range(HQ_):
        sc = q[:, m] @ k.T / math.sqrt(D_)
        sc = np.where(causal, sc, -np.inf)
        sc = sc - sc.max(axis=-1, keepdims=True)
        p = np.exp(sc)
        p /= p.sum(axis=-1, keepdims=True)
        out[:, m * D_:(m + 1) * D_] = p @ v
    y = out @ woT.astype(np.float64)  # [S, E]
    return np.ascontiguousarray(y.T.astype(np.float32))  # yT


# ---------------------------------------------------------------------------
# Entry point
# ---------------------------------------------------------------------------

_NC_CACHE = {}


def _get_nc():
    if "nc" not in _NC_CACHE:
        _NC_CACHE["nc"] = build_nc()
    return _NC_CACHE["nc"]


def kernel(x, wq_w, wk_w, wv_w, wo_w, freqs_cos, freqs_sin, start_pos=0,
           **_ignored):
    from concourse.bass_utils import run_bass_kernel_spmd

    nc = _get_nc()
    in_maps = shard_inputs(x, wq_w, wk_w, wv_w, wo_w, freqs_cos, freqs_sin)
    res = run_bass_kernel_spmd(nc, in_maps, list(range(N_CORES)))
    return gather_output(res.results)



# revision 42
# speedup vs baseline: 1.1133x; 1.1133x over previous
"""Bass/Tile Trainium2 kernel for nn_Attention_37538014167301.

GQA attention layer (B=2, S=2048, E=2048, H=16, KVH=4, D=128) with RoPE and
causal softmax, sharded over 8 NeuronCores: batch (2-way) x head-group
(4-way tensor parallel).  Core c handles batch b=c//4 and q heads
{g, g+4, g+8, g+12} with g=c%4; under torch-style .repeat (jnp.tile) GQA,
those q heads all use kv head g, so each core needs exactly one kv head.

Everything on device is kept in transposed [dim, seq] layout so that every
matmul contracts over the partition axis:
  - projections compute Q^T/K^T/V^T = W @ x^T directly (bf16 inputs,
    fp32 PSUM accumulation),
  - RoPE is applied in [d, s] layout using a pair-swap permutation matmul
    plus elementwise ops against host-precomputed cos/sin tables,
  - scores are computed transposed (P^T[sk, sq]) in fp32r so that P^T can
    feed the attention*V matmul (bf16) with V in natural [sk, d] layout,
  - the softmax denominator is accumulated on DVE in bf16 (2x mode) and
    reduced across partitions with one ones-matmul per (pair, chunk),
  - the output projection (bf16) accumulates y^T[f, s] per core; the host
    sums the four head-group partials per batch.

v3 (emit_attention_v3) splits PSUM into three independent rings
(projections | scores | attention accumulators) so the Tile scheduler can
overlap next-chunk projections and the previous chunk's output projection
with the exp-latency-bound attention loop.  The timing loop (reps>1) uses
For_i with staggered semaphore reset + all-engine branch hints.
"""

import math

import numpy as np

B, S, E = 2, 2048, 2048
H, KVH = 16, 4
D = E // H  # 128
N_CORES = 8
GROUPS = 4  # head groups (tensor-parallel degree per batch)
HQ = H // GROUPS  # q heads per core


# ---------------------------------------------------------------------------
# Device program
# ---------------------------------------------------------------------------

def emit_attention(tc, io, S_=S, E_=E, HQ_=HQ, D_=D, CH=512, XSPLIT=4):
    """Emit the per-core attention program into TileContext tc.

    io: dict of DRAM APs: xT, wqT, wkT, wvT, woT, cosd, sind, swp, yT
    """
    import concourse.mybir as mybir
    from concourse.masks import make_identity

    nc = tc.nc
    f32 = mybir.dt.float32
    f32r = mybir.dt.float32r

    NE = E_ // 128        # contraction tiles over e
    XSPLIT = min(XSPLIT, NE)
    NSQ = S_ // CH        # q chunks
    R = CH // 128         # sk tiles per q chunk width
    NSK = S_ // 128       # sk tiles
    DQ = HQ_ * D_
    scale = 1.0 / math.sqrt(D_)

    xT, wqT, wkT, wvT, woT = io["xT"], io["wqT"], io["wkT"], io["wvT"], io["woT"]
    cosd, sind, swp, yT = io["cosd"], io["sind"], io["swp"], io["yT"]

    from contextlib import ExitStack

    with ExitStack() as ctx:
        const = ctx.enter_context(tc.tile_pool(name="const", bufs=1))
        persist = ctx.enter_context(tc.tile_pool(name="persist", bufs=1))
        pacc = ctx.enter_context(tc.tile_pool(name="pacc", bufs=2, space="PSUM"))
        pden = ctx.enter_context(tc.tile_pool(name="pden", bufs=2, space="PSUM"))

        cos_sb = const.tile([D_, S_], f32, name="cos_sb")
        nc.sync.dma_start(out=cos_sb, in_=cosd)
        sin_sb = const.tile([D_, S_], f32, name="sin_sb")
        nc.sync.dma_start(out=sin_sb, in_=sind)
        swp_sb = const.tile([D_, D_], f32r, name="swp_sb")
        nc.sync.dma_start(out=swp_sb, in_=swp)
        ones_f = const.tile([128, 1], f32, name="ones_f")
        nc.gpsimd.memset(ones_f, 1.0)
        ones_sb = const.tile([128, 1], f32r, name="ones_sb")
        nc.vector.tensor_copy(out=ones_sb, in_=ones_f)
        ident_sb = const.tile([128, 128], f32, name="ident_sb")
        make_identity(nc, ident_sb)

        q_rot = [persist.tile([D_, S_], f32r, name=f"qrot{m}") for m in range(HQ_)]
        k_rot = persist.tile([D_, S_], f32r, name="k_rot")
        v_nat = persist.tile([128, NSK, D_], f32r, name="v_nat")

        # ------------------------------------------------------------------
        # Phase A: projections + rope, chunk by chunk over s
        # ------------------------------------------------------------------
        with ExitStack() as actx:
            wpool = actx.enter_context(tc.tile_pool(name="wpool", bufs=1))
            xpool = actx.enter_context(tc.tile_pool(name="xpool", bufs=2 * XSPLIT))
            work = actx.enter_context(tc.tile_pool(name="awork", bufs=3))
            pmm = actx.enter_context(tc.tile_pool(name="pmm", bufs=4, space="PSUM"))

            # weights arrive as [E, n] = [(t p), n] -> [p, t, n] sbuf layout.
            # Small K/V weights first so the first projection group's inputs
            # (wk + x chunk 0) aren't stuck behind the 4MB wq transfer.
            wk_sb = wpool.tile([128, NE, D_], f32r, name="wk_sb")
            nc.sync.dma_start(out=wk_sb,
                              in_=wkT.rearrange("(t p) n -> p t n", p=128))
            wv_sb = wpool.tile([128, NE, D_], f32r, name="wv_sb")
            nc.sync.dma_start(out=wv_sb,
                              in_=wvT.rearrange("(t p) n -> p t n", p=128))

            xT_r = xT.rearrange("(t p) s -> p t s", p=128)  # [128, NE, S]
            TG = NE // XSPLIT  # e-tiles per x DMA
            wq_sb = wpool.tile([128, NE, DQ], f32r, name="wq_sb")
            wqT_r = wqT.rearrange("(t p) n -> p t n", p=128)

            for j in range(NSQ):
                ssl = slice(CH * j, CH * (j + 1))
                x_chunks = []
                for u in range(XSPLIT):
                    xc = xpool.tile([128, TG, CH], f32r, name="xc", tag="xc")
                    nc.sync.dma_start(
                        out=xc, in_=xT_r[:, TG * u:TG * (u + 1), ssl])
                    x_chunks.append(xc)
                x_tiles = [x_chunks[t // TG][:, t % TG, :] for t in range(NE)]
                if j == 0:
                    # wq arrives after the first x chunk, in 4 e-tile groups
                    for u in range(XSPLIT):
                        nc.sync.dma_start(
                            out=wq_sb[:, TG * u:TG * (u + 1), :],
                            in_=wqT_r[:, TG * u:TG * (u + 1), :])

                def project(w_slices, n):
                    ps = pmm.tile([128, CH], f32, name="ps_proj", tag="mm")
                    for t in range(NE):
                        nc.tensor.matmul(
                            ps[:n, :], w_slices(t), x_tiles[t],
                            start=(t == 0), stop=(t == NE - 1),
                        )
                    return ps

                def rope(ps, dst):
                    # dst[:, ssl] = ps * cos + (SWAP @ ps) * sin_signed
                    p_sb = work.tile([D_, CH], f32r, name="rope_in", tag="rope_in")
                    nc.vector.tensor_copy(out=p_sb, in_=ps[:D_, :])
                    ps2 = pmm.tile([128, CH], f32, name="ps_swap", tag="mm")
                    nc.tensor.matmul(ps2[:D_, :], swp_sb, p_sb,
                                     start=True, stop=True)
                    t1 = work.tile([D_, CH], f32, name="rope_t1", tag="rope_t1")
                    nc.vector.tensor_mul(t1, p_sb.bitcast(f32), cos_sb[:, ssl])
                    t2 = work.tile([D_, CH], f32, name="rope_t2", tag="rope_t2")
                    nc.vector.tensor_mul(t2, ps2[:D_, :], sin_sb[:, ssl])
                    nc.vector.tensor_add(dst[:, ssl], t1, t2)

                # K
                ps = project(lambda t: wk_sb[:, t, :], D_)
                rope(ps, k_rot)
                # V: copy to sbuf, then PE-transpose each 128x128 block
                ps = project(lambda t: wv_sb[:, t, :], D_)
                vt_sb = work.tile([D_, CH], f32, name="vt_sb", tag="vt_sb")
                nc.vector.tensor_copy(out=vt_sb, in_=ps[:D_, :])
                for u in range(R):
                    pvt = pmm.tile([128, CH], f32, name="ps_vt", tag="mm")
                    nc.tensor.transpose(pvt[:, :D_], vt_sb[:, 128 * u:128 * (u + 1)],
                                        ident_sb)
                    nc.vector.tensor_copy(out=v_nat[:, R * j + u, :], in_=pvt[:, :D_])
                # Q heads
                for m in range(HQ_):
                    ps = project(lambda t: wq_sb[:, t, D_ * m:D_ * (m + 1)], D_)
                    rope(ps, q_rot[m])

        # ------------------------------------------------------------------
        # Phase B: attention per (head, q chunk); Phase C: output projection
        # ------------------------------------------------------------------
        with ExitStack() as bctx:
            bpool = bctx.enter_context(tc.tile_pool(name="bpool", bufs=1))
            pwork = bctx.enter_context(tc.tile_pool(name="pwork", bufs=4))
            nwork = bctx.enter_context(tc.tile_pool(name="nwork", bufs=2))
            psc_pool = bctx.enter_context(
                tc.tile_pool(name="psc", bufs=2, space="PSUM"))

            o_sb = [bpool.tile([D_, S_], f32r, name=f"osb{m}") for m in range(HQ_)]

            wopool = bctx.enter_context(tc.tile_pool(name="wopool", bufs=1))
            ywork = bctx.enter_context(tc.tile_pool(name="ywork", bufs=3))
            wo_sb = wopool.tile([128, HQ_, E_], f32r, name="wo_sb")
            for m in range(HQ_):
                nc.sync.dma_start(out=wo_sb[:, m, :], in_=woT[128 * m:128 * (m + 1), :])

            assert HQ_ % 2 == 0
            for j in range(NSQ):
                ssl = slice(CH * j, CH * (j + 1))
                for hp in range(HQ_ // 2):
                    m0, m1 = 2 * hp, 2 * hp + 1
                    n_i = R * (j + 1)
                    po0 = pacc.tile([128, CH], f32, name="po0", tag="acc")
                    po1 = pacc.tile([128, CH], f32, name="po1", tag="acc")
                    pd0 = pden.tile([1, CH], f32, name="pd0", tag="den")
                    pd1 = pden.tile([1, CH], f32, name="pd1", tag="den")
                    for i in range(n_i):
                        # diagonal tiles: columns < 128*u are fully below the
                        # causal boundary -> skip them in scores/exp/PV/den;
                        # only the 128-wide strip [128u, 128(u+1)) needs a mask
                        diag = i >= R * j
                        u = i - R * j if diag else 0
                        lo = 128 * u
                        csl = slice(lo, CH)
                        qsl = slice(CH * j + lo, CH * (j + 1))
                        # transposed scores for both heads into one 2-bank psum
                        psc = psc_pool.tile([128, 2, CH], f32, name="psc",
                                            tag="sc")
                        nc.tensor.matmul(
                            psc[:, 0, csl], k_rot[:, 128 * i:128 * (i + 1)],
                            q_rot[m0][:, qsl], start=True, stop=True)
                        nc.tensor.matmul(
                            psc[:, 1, csl], k_rot[:, 128 * i:128 * (i + 1)],
                            q_rot[m1][:, qsl], start=True, stop=True)
                        p_sb = pwork.tile([128, 2, CH], f32r, name="p_sb",
                                          tag="p_sb")
                        nc.scalar.activation(out=p_sb[:, :, csl],
                                             in_=psc[:, :, csl],
                                             func=mybir.ActivationFunctionType.Exp,
                                             scale=scale)
                        if diag:
                            # keep where sk <= sq, i.e. strip col c' >= p
                            nc.gpsimd.affine_select(
                                out=p_sb[:, :, lo:lo + 128],
                                in_=p_sb[:, :, lo:lo + 128],
                                compare_op=mybir.AluOpType.is_ge,
                                fill=0.0,
                                base=0,
                                pattern=[[0, 2], [1, 128]],
                                channel_multiplier=-1,
                            )
                        last = i == n_i - 1
                        nc.tensor.matmul(po0[:, csl], v_nat[:, i, :],
                                         p_sb[:, 0, csl],
                                         start=(i == 0), stop=last)
                        nc.tensor.matmul(po1[:, csl], v_nat[:, i, :],
                                         p_sb[:, 1, csl],
                                         start=(i == 0), stop=last)
                        nc.tensor.matmul(pd0[:, csl], ones_sb,
                                         p_sb[:, 0, csl],
                                         start=(i == 0), stop=last)
                        nc.tensor.matmul(pd1[:, csl], ones_sb,
                                         p_sb[:, 1, csl],
                                         start=(i == 0), stop=last)
                    for mm, po, pd in ((m0, po0, pd0), (m1, po1, pd1)):
                        # copy unnormalized out to sbuf on ACT so the psum
                        # accumulator frees without waiting for the
                        # recip/broadcast chain
                        ou = nwork.tile([D_, CH], f32, name="ou", tag="ou")
                        nc.scalar.activation(
                            out=ou, in_=po[:D_, :],
                            func=mybir.ActivationFunctionType.Copy)
                        recip = nwork.tile([1, CH], f32, name="recip",
                                           tag="recip")
                        nc.vector.reciprocal(out=recip, in_=pd)
                        rbc = nwork.tile([128, CH], f32, name="rbc", tag="rbc")
                        nc.gpsimd.partition_broadcast(rbc, recip)
                        nc.vector.tensor_mul(o_sb[mm][:, ssl], ou,
                                             rbc[:D_, :])

                # output projection for this s-chunk (pipelines behind
                # attention of the next chunk)
                for tf in range(NE):
                    fsl = slice(128 * tf, 128 * (tf + 1))
                    py = pacc.tile([128, CH], f32, name="py", tag="acc")
                    for m in range(HQ_):
                        nc.tensor.matmul(py, wo_sb[:, m, fsl],
                                         o_sb[m][:, ssl],
                                         start=(m == 0), stop=(m == HQ_ - 1))
                    y_sb = ywork.tile([128, CH], f32, name="y_sb", tag="y_sb")
                    nc.any.tensor_copy(out=y_sb, in_=py)
                    nc.sync.dma_start(out=yT[fsl, ssl], in_=y_sb)


def emit_attention_v2(tc, io, S_=S, E_=E, HQ_=HQ, D_=D, CH=512, XSPLIT=8,
                      phases="ABC"):
    """Fused emission: per s-chunk, projections + rope + attention are
    interleaved so PE has dense work across what were phase boundaries.
    Output projection stays a final phase (SBUF can't hold wo alongside the
    projection working set).

    PSUM budget (8 banks): psc pool 2 bufs x [128,2,CH] (4) + pacc 2 + pden 2.
    Projection accumulators, rope-swap outputs and V-transposes share psc
    slots in head pairs.
    """
    import concourse.mybir as mybir
    from concourse.masks import make_identity

    nc = tc.nc
    f32 = mybir.dt.float32
    f32r = mybir.dt.float32r

    NE = E_ // 128
    XSPLIT = min(XSPLIT, NE)
    NSQ = S_ // CH
    R = CH // 128
    NSK = S_ // 128
    DQ = HQ_ * D_
    NP = HQ_ // 2  # head pairs
    scale = 1.0 / math.sqrt(D_)
    assert HQ_ % 2 == 0

    xT, wqT, wkT, wvT, woT = io["xT"], io["wqT"], io["wkT"], io["wvT"], io["woT"]
    cosd, sind, swp, yT = io["cosd"], io["sind"], io["swp"], io["yT"]

    from contextlib import ExitStack

    with ExitStack() as ctx:
        const = ctx.enter_context(tc.tile_pool(name="const", bufs=1))
        persist = ctx.enter_context(tc.tile_pool(name="persist", bufs=1))
        qpool = ctx.enter_context(tc.tile_pool(name="qpool", bufs=2))
        cpool = ctx.enter_context(tc.tile_pool(name="cpool", bufs=2))
        pacc = ctx.enter_context(tc.tile_pool(name="pacc", bufs=2, space="PSUM"))
        pden = ctx.enter_context(tc.tile_pool(name="pden", bufs=2, space="PSUM"))
        psc_pool = ctx.enter_context(
            tc.tile_pool(name="psc", bufs=2, space="PSUM"))

        swp_sb = const.tile([D_, D_], f32r, name="swp_sb")
        nc.sync.dma_start(out=swp_sb, in_=swp)
        ones_f = const.tile([128, 1], f32, name="ones_f")
        nc.gpsimd.memset(ones_f, 1.0)
        ones_sb = const.tile([128, 1], f32r, name="ones_sb")
        nc.vector.tensor_copy(out=ones_sb, in_=ones_f)
        ident_sb = const.tile([128, 128], f32, name="ident_sb")
        make_identity(nc, ident_sb)

        k_rot = persist.tile([D_, S_], f32r, name="k_rot")
        v_nat = persist.tile([128, NSK, D_], f32r, name="v_nat")
        o_sb = [persist.tile([D_, S_], f32r, name=f"osb{m}")
                for m in range(HQ_)]

        with ExitStack() as actx:
            wpool = actx.enter_context(tc.tile_pool(name="wpool", bufs=1))
            xpool = actx.enter_context(tc.tile_pool(name="xpool", bufs=XSPLIT))
            work = actx.enter_context(tc.tile_pool(name="awork", bufs=2))
            pwork = actx.enter_context(tc.tile_pool(name="pwork", bufs=3))
            nwork = actx.enter_context(tc.tile_pool(name="nwork", bufs=2))

            wk_sb = wpool.tile([128, NE, D_], f32r, name="wk_sb")
            nc.sync.dma_start(out=wk_sb,
                              in_=wkT.rearrange("(t p) n -> p t n", p=128))
            wv_sb = wpool.tile([128, NE, D_], f32r, name="wv_sb")
            nc.sync.dma_start(out=wv_sb,
                              in_=wvT.rearrange("(t p) n -> p t n", p=128))

            xT_r = xT.rearrange("(t p) s -> p t s", p=128)
            TG = NE // XSPLIT
            wq_sb = wpool.tile([128, NE, DQ], f32r, name="wq_sb")
            wqT_r = wqT.rearrange("(t p) n -> p t n", p=128)

            for j in range(NSQ):
                ssl = slice(CH * j, CH * (j + 1))
                x_chunks = []
                for u in range(XSPLIT):
                    xc = xpool.tile([128, TG, CH], f32r, name="xc", tag="xc")
                    nc.sync.dma_start(
                        out=xc, in_=xT_r[:, TG * u:TG * (u + 1), ssl])
                    x_chunks.append(xc)
                x_tiles = [x_chunks[t // TG][:, t % TG, :] for t in range(NE)]
                if j == 0:
                    for u in range(XSPLIT):
                        nc.sync.dma_start(
                            out=wq_sb[:, TG * u:TG * (u + 1), :],
                            in_=wqT_r[:, TG * u:TG * (u + 1), :])

                cos_c = cpool.tile([D_, CH], f32, name="cos_c", tag="cos_c")
                nc.sync.dma_start(out=cos_c, in_=cosd[:, ssl])
                sin_c = cpool.tile([D_, CH], f32, name="sin_c", tag="sin_c")
                nc.sync.dma_start(out=sin_c, in_=sind[:, ssl])

                # --- K+V projections into one paired psum slot ---
                pkv = psc_pool.tile([128, 2, CH], f32, name="pkv", tag="sc")
                for t in range(NE):
                    nc.tensor.matmul(pkv[:, 0, :], wk_sb[:, t, :], x_tiles[t],
                                     start=(t == 0), stop=(t == NE - 1))
                for t in range(NE):
                    nc.tensor.matmul(pkv[:, 1, :], wv_sb[:, t, :], x_tiles[t],
                                     start=(t == 0), stop=(t == NE - 1))

                # --- K rope + V transpose reuse the pkv slot banks ---
                rink = work.tile([D_, 2, CH], f32r, name="rin", tag="rin")
                nc.vector.tensor_copy(out=rink[:, 0, :], in_=pkv[:, 0, :])
                nc.tensor.matmul(pkv[:, 0, :], swp_sb, rink[:, 0, :],
                                 start=True, stop=True)
                t1k = work.tile([D_, 2, CH], f32, name="t1", tag="t1")
                nc.vector.tensor_mul(t1k[:, 0, :], rink[:, 0, :].bitcast(f32),
                                     cos_c)
                t2k = work.tile([D_, 2, CH], f32, name="t2", tag="t2")
                nc.vector.tensor_mul(t2k[:, 0, :], pkv[:, 0, :], sin_c)
                nc.vector.tensor_add(k_rot[:, ssl], t1k[:, 0, :], t2k[:, 0, :])

                vt_sb = work.tile([D_, CH], f32, name="vt_sb", tag="vt_sb")
                nc.vector.tensor_copy(out=vt_sb, in_=pkv[:, 1, :])
                for u in range(R):
                    nc.tensor.transpose(pkv[:, 1, 128 * u:128 * (u + 1)],
                                        vt_sb[:, 128 * u:128 * (u + 1)],
                                        ident_sb)
                    nc.vector.tensor_copy(out=v_nat[:, R * j + u, :],
                                          in_=pkv[:, 1, 128 * u:128 * (u + 1)])

                # --- Q projections + rope, in head pairs ---
                qp = []
                for p in range(NP):
                    m0 = 2 * p
                    pq = psc_pool.tile([128, 2, CH], f32, name="pq", tag="sc")
                    for h in range(2):
                        wsl = slice(D_ * (m0 + h), D_ * (m0 + h + 1))
                        for t in range(NE):
                            nc.tensor.matmul(
                                pq[:, h, :], wq_sb[:, t, wsl], x_tiles[t],
                                start=(t == 0), stop=(t == NE - 1))
                    rin = work.tile([D_, 2, CH], f32r, name="rin", tag="rin")
                    nc.vector.tensor_copy(out=rin, in_=pq[:D_, :, :])
                    for h in range(2):
                        nc.tensor.matmul(pq[:D_, h, :], swp_sb, rin[:, h, :],
                                         start=True, stop=True)
                    cos_b = cos_c[:, None, :].broadcast_to([D_, 2, CH])
                    sin_b = sin_c[:, None, :].broadcast_to([D_, 2, CH])
                    t1 = work.tile([D_, 2, CH], f32, name="t1", tag="t1")
                    nc.vector.tensor_mul(t1, rin.bitcast(f32), cos_b)
                    t2 = work.tile([D_, 2, CH], f32, name="t2", tag="t2")
                    nc.vector.tensor_mul(t2, pq[:D_, :, :], sin_b)
                    qrot = qpool.tile([D_, 2, CH], f32r, name=f"qrot{p}",
                                      tag=f"qrot{p}")
                    nc.vector.tensor_add(qrot, t1, t2)
                    qp.append(qrot)

                # --- attention for this chunk ---
                for p in (range(NP) if "B" in phases else ()):
                    m0, m1 = 2 * p, 2 * p + 1
                    n_i = R * (j + 1)
                    po0 = pacc.tile([128, CH], f32, name="po0", tag="acc")
                    po1 = pacc.tile([128, CH], f32, name="po1", tag="acc")
                    pd0 = pden.tile([1, CH], f32, name="pd0", tag="den")
                    pd1 = pden.tile([1, CH], f32, name="pd1", tag="den")
                    for i in range(n_i):
                        diag = i >= R * j
                        u = i - R * j if diag else 0
                        lo = 128 * u
                        csl = slice(lo, CH)
                        W = CH - lo
                        # merge the head pair into one matmul when the
                        # combined moving size fits the 512 fp32 limit
                        merged = False  # CoreSim can't validate strided pair matmuls
                        psc = psc_pool.tile([128, 2, CH], f32, name="psc",
                                            tag="sc")
                        ksl = k_rot[:, 128 * i:128 * (i + 1)]
                        if merged:
                            nc.tensor.matmul(psc[:, :, csl], ksl,
                                             qp[p][:, :, csl],
                                             start=True, stop=True)
                        else:
                            nc.tensor.matmul(psc[:, 0, csl], ksl,
                                             qp[p][:, 0, csl],
                                             start=True, stop=True)
                            nc.tensor.matmul(psc[:, 1, csl], ksl,
                                             qp[p][:, 1, csl],
                                             start=True, stop=True)
                        p_sb = pwork.tile([128, 2, CH], f32r, name="p_sb",
                                          tag="p_sb")
                        nc.scalar.activation(
                            out=p_sb[:, :, csl], in_=psc[:, :, csl],
                            func=mybir.ActivationFunctionType.Exp, scale=scale)
                        if diag:
                            nc.gpsimd.affine_select(
                                out=p_sb[:, :, lo:lo + 128],
                                in_=p_sb[:, :, lo:lo + 128],
                                compare_op=mybir.AluOpType.is_ge,
                                fill=0.0, base=0,
                                pattern=[[0, 2], [1, 128]],
                                channel_multiplier=-1,
                            )
                        last = i == n_i - 1
                        if merged:
                            nc.tensor.matmul(po0[:, csl], v_nat[:, i, :],
                                             p_sb[:, 0, csl],
                                             start=(i == 0), stop=last)
                            nc.tensor.matmul(pd0[:, csl], ones_sb,
                                             p_sb[:, 0, csl],
                                             start=(i == 0), stop=last)
                            nc.tensor.matmul(pd1[:, csl], ones_sb,
                                             p_sb[:, 1, csl],
                                             start=(i == 0), stop=last)
                        else:
                            # stop=True closes each matmul's psum group so the
                            # paired po tile never has two pending groups;
                            # has_written persists, so accumulation continues
                            nc.tensor.matmul(po0[:, csl], v_nat[:, i, :],
                                             p_sb[:, 0, csl],
                                             start=(i == 0), stop=last)
                            nc.tensor.matmul(po1[:, csl], v_nat[:, i, :],
                                             p_sb[:, 1, csl],
                                             start=(i == 0), stop=last)
                            nc.tensor.matmul(pd0[:, csl], ones_sb,
                                             p_sb[:, 0, csl],
                                             start=(i == 0), stop=last)
                            nc.tensor.matmul(pd1[:, csl], ones_sb,
                                             p_sb[:, 1, csl],
                                             start=(i == 0), stop=last)
                    for mm, po, pd in ((m0, po0, pd0), (m1, po1, pd1)):
                        ou = nwork.tile([D_, CH], f32, name="ou", tag="ou")
                        nc.scalar.activation(
                            out=ou, in_=po[:D_, :],
                            func=mybir.ActivationFunctionType.Copy)
                        recip = nwork.tile([1, CH], f32, name="recip",
                                           tag="recip")
                        nc.vector.reciprocal(out=recip, in_=pd)
                        rbc = nwork.tile([128, CH], f32, name="rbc", tag="rbc")
                        nc.gpsimd.partition_broadcast(rbc, recip)
                        nc.vector.tensor_mul(o_sb[mm][:, ssl], ou,
                                             rbc[:D_, :])

        # --- output projection (phase C) ---
        if "C" not in phases:
            return
        with ExitStack() as cctx:
            wopool = cctx.enter_context(tc.tile_pool(name="wopool", bufs=1))
            ywork = cctx.enter_context(tc.tile_pool(name="ywork", bufs=3))
            wo_sb = wopool.tile([128, HQ_, E_], f32r, name="wo_sb")
            for m in range(HQ_):
                nc.sync.dma_start(out=wo_sb[:, m, :],
                                  in_=woT[128 * m:128 * (m + 1), :])
            for tf in range(NE):
                fsl = slice(128 * tf, 128 * (tf + 1))
                for j in range(NSQ):
                    ssl = slice(CH * j, CH * (j + 1))
                    py = pacc.tile([128, CH], f32, name="py", tag="acc")
                    for m in range(HQ_):
                        nc.tensor.matmul(py, wo_sb[:, m, fsl],
                                         o_sb[m][:, ssl],
                                         start=(m == 0), stop=(m == HQ_ - 1))
                    y_sb = ywork.tile([128, CH], f32, name="y_sb", tag="y_sb")
                    nc.any.tensor_copy(out=y_sb, in_=py)
                    nc.sync.dma_start(out=yT[fsl, ssl], in_=y_sb)


def emit_attention_v3(tc, io, S_=S, E_=E, HQ_=HQ, D_=D, CH=512, XSPLIT=8,
                      merge_strip=True):
    """v3: scheduler-friendly restructure of v2.

    - PSUM split into three independent rings so the Tile scheduler can run
      projections (next chunk), attention (current chunk) and the output
      projection (current chunk) concurrently:
        pproj 2x[128,CH] (2 banks) | psc 2x[128,2,CH] (4) | pacc 2x[128,CH] (2)
    - softmax denominator accumulated on DVE in bf16 (2x mode) instead of
      per-tile PE matmuls; a single ones-matmul per (pair, chunk) does the
      final 128-partition reduction.
    - p_sb / V / o / wo / y in bf16: halves DVE+DMA cost, PE rate unchanged.
    - rope PSUM->SBUF copies on ACT, cos-muls on GpSimd, sin-muls + final
      adds on DVE (spreads the elementwise load off DVE).
    - output projection emitted per chunk so it fills PE gaps left by the
      exp-latency-bound attention loop.
    """
    import concourse.mybir as mybir
    from concourse.masks import make_identity

    nc = tc.nc
    f32 = mybir.dt.float32
    f32r = mybir.dt.float32r
    bf16 = mybir.dt.bfloat16

    NE = E_ // 128
    XSPLIT = min(XSPLIT, NE)
    NSQ = S_ // CH
    R = CH // 128
    NSK = S_ // 128
    DQ = HQ_ * D_
    NP = HQ_ // 2
    scale = 1.0 / math.sqrt(D_)
    assert HQ_ % 2 == 0

    xT, wqT, wkT, wvT, woT = io["xT"], io["wqT"], io["wkT"], io["wvT"], io["woT"]
    cosd, sind, swp, yT = io["cosd"], io["sind"], io["swp"], io["yT"]

    from contextlib import ExitStack

    with ExitStack() as ctx:
        const = ctx.enter_context(tc.tile_pool(name="const", bufs=1))
        persist = ctx.enter_context(tc.tile_pool(name="persist", bufs=1))
        wpool = ctx.enter_context(tc.tile_pool(name="wpool", bufs=2))
        xpool = ctx.enter_context(tc.tile_pool(name="xpool", bufs=2 * XSPLIT))
        cpool = ctx.enter_context(tc.tile_pool(name="cpool", bufs=2))
        qpool = ctx.enter_context(tc.tile_pool(name="qpool", bufs=2))
        rwork = ctx.enter_context(tc.tile_pool(name="rwork", bufs=2))
        pwork = ctx.enter_context(tc.tile_pool(name="pwork", bufs=4))
        dpool = ctx.enter_context(tc.tile_pool(name="dpool", bufs=2))
        nwork = ctx.enter_context(tc.tile_pool(name="nwork", bufs=2))
        ywork = ctx.enter_context(tc.tile_pool(name="ywork", bufs=6))
        pproj = ctx.enter_context(tc.tile_pool(name="pproj", bufs=2,
                                               space="PSUM"))
        psc_pool = ctx.enter_context(tc.tile_pool(name="psc", bufs=2,
                                                  space="PSUM"))
        pacc = ctx.enter_context(tc.tile_pool(name="pacc", bufs=2,
                                              space="PSUM"))

        swp_sb = const.tile([D_, D_], f32r, name="swp_sb")
        nc.sync.dma_start(out=swp_sb, in_=swp)
        ones_f = const.tile([128, 1], f32, name="ones_f")
        nc.gpsimd.memset(ones_f, 1.0)
        ones_bf = const.tile([128, 1], bf16, name="ones_bf")
        nc.vector.tensor_copy(out=ones_bf, in_=ones_f)
        ident_f = const.tile([128, 128], f32, name="ident_f")
        make_identity(nc, ident_f)
        ident_bf = const.tile([128, 128], bf16, name="ident_bf")
        nc.vector.tensor_copy(out=ident_bf, in_=ident_f)

        k_rot = persist.tile([D_, S_], f32r, name="k_rot")
        v_nat = persist.tile([128, NSK, D_], bf16, name="v_nat")
        o_sb = [persist.tile([D_, S_], bf16, name=f"osb{m}")
                for m in range(HQ_)]

        # weight tiles; DMA emission is ordered inside chunk 0 so the serial
        # DMA stream matches first-use order: cos/sin, wk, x, wv, wq, wo
        wk_sb = wpool.tile([128, NE, D_], bf16, name="wk_sb")
        wv_sb = wpool.tile([128, NE, D_], bf16, name="wv_sb")
        wo_sb = wpool.tile([128, HQ_, E_], bf16, name="wo_sb")

        xT_r = xT.rearrange("(t p) s -> p t s", p=128)
        TG = NE // XSPLIT
        wq_sb = wpool.tile([128, NE, DQ], bf16, name="wq_sb")
        wqT_r = wqT.rearrange("(t p) n -> p t n", p=128)

        for j in range(NSQ):
            ssl = slice(CH * j, CH * (j + 1))
            cos_c = cpool.tile([D_, CH], f32, name="cos_c", tag="cos_c")
            nc.sync.dma_start(out=cos_c, in_=cosd[:, ssl])
            sin_c = cpool.tile([D_, CH], f32, name="sin_c", tag="sin_c")
            nc.sync.dma_start(out=sin_c, in_=sind[:, ssl])
            if j == 0:
                nc.sync.dma_start(out=wk_sb,
                                  in_=wkT.rearrange("(t p) n -> p t n", p=128))
            x_chunks = []
            for u in range(XSPLIT):
                xc = xpool.tile([128, TG, CH], bf16, name="xc", tag="xc")
                nc.sync.dma_start(out=xc, in_=xT_r[:, TG * u:TG * (u + 1), ssl])
                x_chunks.append(xc)
            x_tiles = [x_chunks[t // TG][:, t % TG, :] for t in range(NE)]
            if j == 0:
                nc.sync.dma_start(out=wv_sb,
                                  in_=wvT.rearrange("(t p) n -> p t n", p=128))
                for u in range(XSPLIT):
                    nc.sync.dma_start(
                        out=wq_sb[:, TG * u:TG * (u + 1), :],
                        in_=wqT_r[:, TG * u:TG * (u + 1), :])
                for m in range(HQ_):
                    nc.sync.dma_start(out=wo_sb[:, m, :],
                                      in_=woT[128 * m:128 * (m + 1), :])

            # ---- K projection + rope ----
            pk = pproj.tile([128, CH], f32, name="pk", tag="proj")
            for t in range(NE):
                nc.tensor.matmul(pk[:D_, :], wk_sb[:, t, :], x_tiles[t],
                                 start=(t == 0), stop=(t == NE - 1))
            krin = rwork.tile([D_, CH], f32r, name="krin", tag="krin", bufs=1)
            nc.scalar.activation(out=krin, in_=pk[:D_, :],
                                 func=mybir.ActivationFunctionType.Copy)
            nc.tensor.matmul(pk[:D_, :], swp_sb, krin, start=True, stop=True)
            kt1 = rwork.tile([D_, CH], f32, name="kt1", tag="kt1", bufs=1)
            nc.gpsimd.tensor_mul(kt1, krin.bitcast(f32), cos_c)
            kt2 = rwork.tile([D_, CH], f32, name="kt2", tag="kt2", bufs=1)
            nc.vector.tensor_mul(kt2, pk[:D_, :], sin_c)
            nc.vector.tensor_add(k_rot[:, ssl], kt1, kt2)

            # ---- V projection + transpose ----
            pv = pproj.tile([128, CH], f32, name="pv", tag="proj")
            for t in range(NE):
                nc.tensor.matmul(pv[:D_, :], wv_sb[:, t, :], x_tiles[t],
                                 start=(t == 0), stop=(t == NE - 1))
            vt_sb = rwork.tile([D_, CH], bf16, name="vt_sb", tag="vt_sb")
            nc.scalar.activation(out=vt_sb, in_=pv[:D_, :],
                                 func=mybir.ActivationFunctionType.Copy)
            pvb = pv.bitcast(bf16)  # reuse the bank as bf16 transpose scratch
            for u in range(R):
                nc.tensor.transpose(pvb[:, 128 * u:128 * (u + 1)],
                                    vt_sb[:, 128 * u:128 * (u + 1)], ident_bf)
                nc.vector.tensor_copy(out=v_nat[:, R * j + u, :],
                                      in_=pvb[:, 128 * u:128 * (u + 1)])

            # ---- Q projections + rope, in head pairs ----
            qp = []
            for p in range(NP):
                rin = rwork.tile([D_, 2, CH], f32r, name="rin", tag="rin")
                qt2 = rwork.tile([D_, 2, CH], f32, name="qt2", tag="qt2")
                for h in range(2):
                    m = 2 * p + h
                    pq = pproj.tile([128, CH], f32, name="pq", tag="proj")
                    wsl = slice(D_ * m, D_ * (m + 1))
                    for t in range(NE):
                        nc.tensor.matmul(pq[:D_, :], wq_sb[:, t, wsl],
                                         x_tiles[t],
                                         start=(t == 0), stop=(t == NE - 1))
                    nc.scalar.activation(
                        out=rin[:, h, :], in_=pq[:D_, :],
                        func=mybir.ActivationFunctionType.Copy)
                    nc.tensor.matmul(pq[:D_, :], swp_sb, rin[:, h, :],
                                     start=True, stop=True)
                    nc.vector.tensor_mul(qt2[:, h, :], pq[:D_, :], sin_c)
                cos_b = cos_c[:, None, :].broadcast_to([D_, 2, CH])
                qt1 = rwork.tile([D_, 2, CH], f32, name="qt1", tag="qt1")
                nc.gpsimd.tensor_mul(qt1, rin.bitcast(f32), cos_b)
                qrot = qpool.tile([D_, 2, CH], f32r, name=f"qrot{p}",
                                  tag=f"qrot{p}")
                nc.vector.tensor_add(qrot, qt1, qt2)
                qp.append(qrot)

            # ---- attention, per head pair ----
            for p in range(NP):
                m0, m1 = 2 * p, 2 * p + 1
                n_i = R * (j + 1)
                po0 = pacc.tile([128, CH], f32, name="po0", tag="acc")
                po1 = pacc.tile([128, CH], f32, name="po1", tag="acc")
                dacc = dpool.tile([128, 2, CH], bf16, name="dacc", tag="dacc")
                for i in range(n_i):
                    diag = i >= R * j
                    u = i - R * j if diag else 0
                    lo = 128 * u
                    csl = slice(lo, CH)
                    psc = psc_pool.tile([128, 2, CH], f32, name="psc",
                                        tag="sc")
                    ksl = k_rot[:, 128 * i:128 * (i + 1)]
                    p_sb = pwork.tile([128, 2, CH], bf16, name="p_sb",
                                      tag="p_sb")
                    if merge_strip and diag and CH - lo == 128:
                        # 128-wide strip: merge the head pair into one matmul
                        # (moving [D,2,128] packed h-major into one bank) to
                        # dodge the fp32r free-dim<256 rate penalty
                        nc.tensor.matmul(psc[:, 0, 0:256], ksl,
                                         qp[p][:, :, csl],
                                         start=True, stop=True)
                        nc.scalar.activation(
                            out=p_sb[:, :, csl], in_=psc[:, 0, 0:256],
                            func=mybir.ActivationFunctionType.Exp,
                            scale=scale)
                    else:
                        nc.tensor.matmul(psc[:, 0, csl], ksl,
                                         qp[p][:, 0, csl],
                                         start=True, stop=True)
                        nc.tensor.matmul(psc[:, 1, csl], ksl,
                                         qp[p][:, 1, csl],
                                         start=True, stop=True)
                        nc.scalar.activation(
                            out=p_sb[:, :, csl], in_=psc[:, :, csl],
                            func=mybir.ActivationFunctionType.Exp,
                            scale=scale)
                    if diag:
                        nc.gpsimd.affine_select(
                            out=p_sb[:, :, lo:lo + 128],
                            in_=p_sb[:, :, lo:lo + 128],
                            compare_op=mybir.AluOpType.is_ge,
                            fill=0.0, base=0,
                            pattern=[[0, 2], [1, 128]],
                            channel_multiplier=-1,
                        )
                    if i == 0:
                        nc.vector.tensor_copy(out=dacc, in_=p_sb)
                    else:
                        nc.vector.tensor_add(dacc[:, :, csl], dacc[:, :, csl],
                                             p_sb[:, :, csl])
                    last = i == n_i - 1
                    nc.tensor.matmul(po0[:, csl], v_nat[:, i, :],
                                     p_sb[:, 0, csl],
                                     start=(i == 0), stop=last)
                    nc.tensor.matmul(po1[:, csl], v_nat[:, i, :],
                                     p_sb[:, 1, csl],
                                     start=(i == 0), stop=last)
                # final denominator reduction + normalization (per head so
                # the recip->broadcast->mul chain pipelines)
                den_t = psc_pool.tile([1, 2, CH], f32, name="den_t", tag="sc")
                for h, (mm, po) in enumerate(((m0, po0), (m1, po1))):
                    nc.tensor.matmul(den_t[:, h, :], ones_bf, dacc[:, h, :],
                                     start=True, stop=True)
                    recip = nwork.tile([1, CH], f32, name="recip",
                                       tag="recip", bufs=2)
                    nc.vector.reciprocal(out=recip, in_=den_t[:, h, :])
                    rbc = nwork.tile([128, CH], f32, name="rbc", tag="rbc")
                    nc.gpsimd.partition_broadcast(rbc, recip)
                    nc.vector.tensor_mul(o_sb[mm][:, ssl], po[:D_, :],
                                         rbc[:D_, :])

            # ---- output projection, one chunk late: emitted after the NEXT
            # chunk's attention in program order, it has higher scheduler
            # priority index and so fills PE stalls left by the exp-latency-
            # bound attention loop of chunk j (which has no proj filler on
            # the last chunk).
            def emit_outproj(jc):
                osl = slice(CH * jc, CH * (jc + 1))
                for tf in range(NE):
                    fsl = slice(128 * tf, 128 * (tf + 1))
                    py = pproj.tile([128, CH], f32, name="py", tag="proj")
                    for m in range(HQ_):
                        nc.tensor.matmul(py, wo_sb[:, m, fsl],
                                         o_sb[m][:, osl],
                                         start=(m == 0), stop=(m == HQ_ - 1))
                    y_sb = ywork.tile([128, CH], bf16, name="y_sb",
                                      tag="y_sb")
                    if tf % 2 == 0:
                        nc.vector.tensor_copy(out=y_sb, in_=py)
                    else:
                        nc.scalar.activation(
                            out=y_sb, in_=py,
                            func=mybir.ActivationFunctionType.Copy)
                    nc.sync.dma_start(out=yT[fsl, osl], in_=y_sb)

            if j > 0:
                emit_outproj(j - 1)
        emit_outproj(NSQ - 1)


def emit_attention_v4(tc, io, S_=S, E_=E, HQ_=HQ, D_=D, CH=512, XSPLIT=4,
                      mid_cb=None):
    """v4: all-bf16 matmul datapath + cheap normalization + no GpSimd libs.

    Changes vs v3 (trace-driven):
    - k_rot / q_rot / rope internals in bf16: the f32r score+swap matmuls
      measured ~2.4x slower than bf16 on HW (84.7us for 172 matmuls).
    - softmax denominator reciprocal via Ln->Exp(-x) on ACT (both funcs in
      the natural_log_exp_and_others table, so no ACT table reloads) and a
      ones-matmul partition broadcast on PE; replaces the [1,CH] DVE
      reciprocal (3.3us each, 53us total) + GpSimd partition_broadcast
      (whose custom-ucode library swaps cost ~4us each).
    - causal diagonal mask via a constant [128,128] triangular bf16 mask
      multiplied on DVE; replaces gpsimd.affine_select.  With
      partition_broadcast also gone, GpSimd runs zero custom libraries.
    - unnormalized attention output copied PSUM->SBUF on DVE right after
      the last PV matmul so the PSUM accumulator frees without waiting
      for the denominator chain.
    - chunk-0 DMA order: wk, x, cos/sin, wv, wq, wo (first-use order).
    - y written back one DMA per 4 f-tiles.
    """
    import concourse.mybir as mybir
    from concourse.masks import make_identity

    nc = tc.nc
    f32 = mybir.dt.float32
    f32r = mybir.dt.float32r
    bf16 = mybir.dt.bfloat16

    NE = E_ // 128
    XSPLIT = min(XSPLIT, NE)
    NSQ = S_ // CH
    R = CH // 128
    NSK = S_ // 128
    DQ = HQ_ * D_
    NP = HQ_ // 2
    scale = 1.0 / math.sqrt(D_)
    assert HQ_ % 2 == 0

    xT, wqT, wkT, wvT, woT = io["xT"], io["wqT"], io["wkT"], io["wvT"], io["woT"]
    cosd, sind, swp, tri, yT = (io["cosd"], io["sind"], io["swp"], io["tri"],
                                io["yT"])

    from contextlib import ExitStack

    with ExitStack() as ctx:
        const = ctx.enter_context(tc.tile_pool(name="const", bufs=1))
        persist = ctx.enter_context(tc.tile_pool(name="persist", bufs=1))
        wpool = ctx.enter_context(tc.tile_pool(name="wpool", bufs=2))
        xpool = ctx.enter_context(tc.tile_pool(name="xpool", bufs=2))
        cpool = ctx.enter_context(tc.tile_pool(name="cpool", bufs=2))
        qpool = ctx.enter_context(tc.tile_pool(name="qpool", bufs=2))
        rwork = ctx.enter_context(tc.tile_pool(name="rwork", bufs=2))
        pwork = ctx.enter_context(tc.tile_pool(name="pwork", bufs=4))
        dpool = ctx.enter_context(tc.tile_pool(name="dpool", bufs=3))
        nwork = ctx.enter_context(tc.tile_pool(name="nwork", bufs=2))
        ywork = ctx.enter_context(tc.tile_pool(name="ywork", bufs=3))
        pproj = ctx.enter_context(tc.tile_pool(name="pproj", bufs=2,
                                               space="PSUM"))
        psc_pool = ctx.enter_context(tc.tile_pool(name="psc", bufs=2,
                                                  space="PSUM"))
        pacc = ctx.enter_context(tc.tile_pool(name="pacc", bufs=2,
                                              space="PSUM"))

        swp_sb = const.tile([D_, D_], bf16, name="swp_sb")
        tri_sb = const.tile([128, 128], bf16, name="tri_sb")
        ones_f = const.tile([128, 1], f32, name="ones_f")
        nc.gpsimd.memset(ones_f, 1.0)
        ones_bf = const.tile([128, 1], bf16, name="ones_bf")
        nc.vector.tensor_copy(out=ones_bf, in_=ones_f)
        onesr_f = const.tile([1, 128], f32, name="onesr_f")
        nc.gpsimd.memset(onesr_f, 1.0)
        onesr_bf = const.tile([1, 128], bf16, name="onesr_bf")
        nc.vector.tensor_copy(out=onesr_bf, in_=onesr_f)
        ident_f = const.tile([128, 128], f32, name="ident_f")
        make_identity(nc, ident_f)
        ident_bf = const.tile([128, 128], bf16, name="ident_bf")
        nc.vector.tensor_copy(out=ident_bf, in_=ident_f)

        k_rot = persist.tile([D_, S_], bf16, name="k_rot")
        v_nat = persist.tile([128, NSK, D_], bf16, name="v_nat")
        o_sb = [persist.tile([D_, S_], bf16, name=f"osb{m}")
                for m in range(HQ_)]

        wk_sb = wpool.tile([128, NE, D_], bf16, name="wk_sb")
        wv_sb = wpool.tile([128, NE, D_], bf16, name="wv_sb")
        wo_sb = wpool.tile([128, HQ_, E_], bf16, name="wo_sb")
        wq_sb = wpool.tile([128, NE, DQ], bf16, name="wq_sb")

        TG = NE // XSPLIT
        yT_r = yT.rearrange("(t p) s -> p t s", p=128)
        xT_r = xT.rearrange("(t p) s -> p t s", p=128)
        wkT_r = wkT.rearrange("(t p) n -> p t n", p=128)
        wqT_r = wqT.rearrange("(t p) n -> p t n", p=128)
        wvT_r = wvT.rearrange("(t p) n -> p t n", p=128)

        # x DMA split sizes: small first group so the first projection
        # matmul of a loop body can start ~2us after the DMA restart
        XSPL = [2, 7, 7] if NE == 16 else [NE // XSPLIT] * XSPLIT
        for j in range(NSQ):
            if j == NSQ // 2 and mid_cb is not None:
                # staggered-reset stage boundary at the body midpoint: the
                # NEXT unrolled body's DMA restart then shares a stage with
                # this body's tail instead of waiting on a stage edge
                mid_cb()
            ssl = slice(CH * j, CH * (j + 1))
            if j == 0:
                nc.sync.dma_start(out=wk_sb[:, 0:2, :], in_=wkT_r[:, 0:2, :])
            x_chunks, x_tiles = [], []
            t_base = 0
            for u, tg in enumerate(XSPL):
                xc = xpool.tile([128, tg, CH], bf16, name="xc", tag=f"xc{u}")
                nc.sync.dma_start(out=xc, in_=xT_r[:, t_base:t_base + tg, ssl])
                x_chunks.append(xc)
                x_tiles.extend(xc[:, i, :] for i in range(tg))
                t_base += tg
                if j == 0 and u == 0:
                    nc.sync.dma_start(out=wk_sb[:, 2:NE, :],
                                      in_=wkT_r[:, 2:NE, :])
            # ACT-sequencer DMAs: Sync's serial issue chain is busy with
            # wk/x at body start; ACT is idle until the first rope copy, so
            # cos/sin arrive ~5us sooner and release the swap-matmul PSUM
            # slot for the Q projections earlier
            cos_c = cpool.tile([D_, CH], bf16, name="cos_c", tag="cos_c")
            nc.scalar.dma_start(out=cos_c, in_=cosd[:, ssl])
            sin_c = cpool.tile([D_, CH], f32, name="sin_c", tag="sin_c")
            nc.scalar.dma_start(out=sin_c, in_=sind[:, ssl])
            if j == 0:
                nc.sync.dma_start(out=swp_sb, in_=swp)
                nc.sync.dma_start(out=tri_sb, in_=tri)
                nc.sync.dma_start(out=wv_sb, in_=wvT_r)
                for u in range(XSPLIT):
                    nc.sync.dma_start(
                        out=wq_sb[:, TG * u:TG * (u + 1), :],
                        in_=wqT_r[:, TG * u:TG * (u + 1), :])
                for m in range(HQ_):
                    nc.sync.dma_start(out=wo_sb[:, m, :],
                                      in_=woT[128 * m:128 * (m + 1), :])

            # ---- K projection + rope ----
            pk = pproj.tile([128, CH], f32, name="pk", tag="proj")
            for t in range(NE):
                nc.tensor.matmul(pk[:D_, :], wk_sb[:, t, :], x_tiles[t],
                                 start=(t == 0), stop=(t == NE - 1))
            krin = rwork.tile([D_, CH], bf16, name="krin", tag="krin", bufs=1)
            nc.scalar.activation(out=krin, in_=pk[:D_, :],
                                 func=mybir.ActivationFunctionType.Copy)
            kt2 = rwork.tile([D_, CH], bf16, name="kt2", tag="kt2", bufs=1)
            nc.tensor.matmul(pk[:D_, :], swp_sb, krin, start=True, stop=True)
            kt1 = rwork.tile([D_, CH], bf16, name="kt1", tag="kt1", bufs=1)
            nc.gpsimd.tensor_mul(kt1, krin, cos_c)
            nc.vector.tensor_mul(kt2, pk[:D_, :], sin_c)
            nc.vector.tensor_add(k_rot[:, ssl], kt1, kt2)

            # ---- V projection + transpose ----
            pv = pproj.tile([128, CH], f32, name="pv", tag="proj")
            for t in range(NE):
                nc.tensor.matmul(pv[:D_, :], wv_sb[:, t, :], x_tiles[t],
                                 start=(t == 0), stop=(t == NE - 1))
            vt_sb = rwork.tile([D_, CH], bf16, name="vt_sb", tag="vt_sb")
            nc.scalar.activation(out=vt_sb, in_=pv[:D_, :],
                                 func=mybir.ActivationFunctionType.Copy)
            pvb = pv.bitcast(bf16)  # reuse the bank as bf16 transpose scratch
            for u in range(R):
                nc.tensor.transpose(pvb[:, 128 * u:128 * (u + 1)],
                                    vt_sb[:, 128 * u:128 * (u + 1)], ident_bf)
                nc.vector.tensor_copy(out=v_nat[:, R * j + u, :],
                                      in_=pvb[:, 128 * u:128 * (u + 1)])

            # ---- Q projections + rope, in head pairs ----
            qp = []
            for p in range(NP):
                rin = rwork.tile([D_, 2, CH], bf16, name="rin", tag="rin")
                qt2 = rwork.tile([D_, 2, CH], bf16, name="qt2", tag="qt2")
                for h in range(2):
                    m = 2 * p + h
                    pq = pproj.tile([128, CH], f32, name="pq", tag="proj")
                    wsl = slice(D_ * m, D_ * (m + 1))
                    for t in range(NE):
                        nc.tensor.matmul(pq[:D_, :], wq_sb[:, t, wsl],
                                         x_tiles[t],
                                         start=(t == 0), stop=(t == NE - 1))
                    nc.scalar.activation(
                        out=rin[:, h, :], in_=pq[:D_, :],
                        func=mybir.ActivationFunctionType.Copy)
                    nc.tensor.matmul(pq[:D_, :], swp_sb, rin[:, h, :],
                                     start=True, stop=True)
                    nc.vector.tensor_mul(qt2[:, h, :], pq[:D_, :], sin_c)
                cos_b = cos_c[:, None, :].broadcast_to([D_, 2, CH])
                qt1 = rwork.tile([D_, 2, CH], bf16, name="qt1", tag="qt1")
                nc.gpsimd.tensor_mul(qt1, rin, cos_b)
                qrot = qpool.tile([D_, 2, CH], bf16, name=f"qrot{p}",
                                  tag=f"qrot{p}")
                nc.vector.tensor_add(qrot, qt1, qt2)
                qp.append(qrot)

            # ---- attention, per head pair ----
            tri_bc = tri_sb[:, None, :].broadcast_to([128, 2, 128])
            pending = []
            for p in range(NP):
                m0, m1 = 2 * p, 2 * p + 1
                n_i = R * (j + 1)
                po0 = pacc.tile([128, CH], f32, name="po0", tag="acc")
                po1 = pacc.tile([128, CH], f32, name="po1", tag="acc")
                dacc = dpool.tile([128, 2, CH], bf16, name="dacc", tag="dacc")
                for i in range(n_i):
                    diag = i >= R * j
                    u = i - R * j if diag else 0
                    lo = 128 * u
                    csl = slice(lo, CH)
                    W = CH - lo
                    psc = psc_pool.tile([128, 2, CH], f32, name="psc",
                                        tag="sc")
                    ksl = k_rot[:, 128 * i:128 * (i + 1)]
                    p_sb = pwork.tile([128, 2, CH], bf16, name="p_sb",
                                      tag="p_sb")
                    if diag and 2 * W <= 512:
                        # merge the head pair into one matmul: moving
                        # [D,2,W] packed h-major into one psum bank
                        nc.tensor.matmul(psc[:, 0, 0:2 * W], ksl,
                                         qp[p][:, :, csl],
                                         start=True, stop=True)
                        nc.scalar.activation(
                            out=p_sb[:, :, csl], in_=psc[:, 0, 0:2 * W],
                            func=mybir.ActivationFunctionType.Exp,
                            scale=scale)
                    else:
                        nc.tensor.matmul(psc[:, 0, csl], ksl,
                                         qp[p][:, 0, csl],
                                         start=True, stop=True)
                        nc.tensor.matmul(psc[:, 1, csl], ksl,
                                         qp[p][:, 1, csl],
                                         start=True, stop=True)
                        nc.scalar.activation(
                            out=p_sb[:, :, csl], in_=psc[:, :, csl],
                            func=mybir.ActivationFunctionType.Exp,
                            scale=scale)
                    if diag:
                        # keep where sk <= sq: strip col c >= row r
                        nc.vector.tensor_mul(p_sb[:, :, lo:lo + 128],
                                             p_sb[:, :, lo:lo + 128],
                                             tri_bc)
                    if i == 0:
                        nc.vector.tensor_copy(out=dacc, in_=p_sb)
                    else:
                        nc.vector.tensor_add(dacc[:, :, csl], dacc[:, :, csl],
                                             p_sb[:, :, csl])
                    last = i == n_i - 1
                    nc.tensor.matmul(po0[:, csl], v_nat[:, i, :],
                                     p_sb[:, 0, csl],
                                     start=(i == 0), stop=last)
                    nc.tensor.matmul(po1[:, csl], v_nat[:, i, :],
                                     p_sb[:, 1, csl],
                                     start=(i == 0), stop=last)
                # copy unnormalized out to SBUF (into o_sb, normalized in
                # place later) so the PSUM accumulators free immediately
                nc.vector.tensor_copy(out=o_sb[m0][:, ssl], in_=po0[:D_, :])
                nc.vector.tensor_copy(out=o_sb[m1][:, ssl], in_=po1[:D_, :])
                pending.append((dacc, m0, m1))

            # ---- output projection, one chunk late (fills PE stalls of
            # the exp-latency-bound attention loop) ----
            def emit_outproj(jc):
                osl = slice(CH * jc, CH * (jc + 1))
                for tg in range(NE // 4):
                    y4 = ywork.tile([128, 4, CH], bf16, name="y4", tag="y4")
                    for tq in range(4):
                        tf = 4 * tg + tq
                        fsl = slice(128 * tf, 128 * (tf + 1))
                        py = pproj.tile([128, CH], f32, name="py", tag="proj")
                        for m in range(HQ_):
                            nc.tensor.matmul(py, wo_sb[:, m, fsl],
                                             o_sb[m][:, osl],
                                             start=(m == 0),
                                             stop=(m == HQ_ - 1))
                        if tq % 2 == 0:
                            nc.vector.tensor_copy(out=y4[:, tq, :], in_=py)
                        else:
                            nc.scalar.activation(
                                out=y4[:, tq, :], in_=py,
                                func=mybir.ActivationFunctionType.Copy)
                    nc.sync.dma_start(out=yT_r[:, 4 * tg:4 * tg + 4, osl],
                                      in_=y4)

            # ---- deferred normalization: den reduce + reciprocal +
            # partition-broadcast + in-place scale.  Emitted after the
            # outproj filler so the PE-stream never stalls on the
            # reciprocal chain; o_sb(j) is only read by outproj(j), one
            # chunk later.
            def emit_norm():
                for dacc_, m0_, m1_ in pending:
                    den_t = psc_pool.tile([1, 2, CH], f32, name="den_t",
                                          tag="sc")
                    for h in range(2):
                        nc.tensor.matmul(den_t[:, h, :], ones_bf,
                                         dacc_[:, h, :],
                                         start=True, stop=True)
                    recip = nwork.tile([1, 2, CH], f32, name="recip",
                                       tag="recip", bufs=2)
                    nc.vector.reciprocal_approx_fast(out=recip, in_=den_t)
                    recip_bf = nwork.tile([1, 2, CH], bf16, name="recip_bf",
                                          tag="recip_bf", bufs=2)
                    nc.vector.tensor_copy(out=recip_bf, in_=recip)
                    for h, mm in ((0, m0_), (1, m1_)):
                        # tag "sc" (not "proj"): borrowing a projection slot
                        # here would block the next chunk's K projection on
                        # this chunk's reciprocal chain
                        pbc = psc_pool.tile([128, CH], f32, name="pbc",
                                            tag="sc")
                        nc.tensor.matmul(pbc, onesr_bf, recip_bf[:, h, :],
                                         start=True, stop=True)
                        nc.vector.tensor_mul(o_sb[mm][:, ssl],
                                             o_sb[mm][:, ssl], pbc[:D_, :])

            if j > 0:
                emit_outproj(j - 1)
            emit_norm()
        emit_outproj(NSQ - 1)


def build_nc(S_=S, E_=E, HQ_=HQ, D_=D, CH=512, n_cores=N_CORES, reps=1,
             version=4, phases="ABC", merge_strip=True):
    """Build and compile the per-core Bass program (same program on all cores).

    reps > 1 wraps the whole body in a hardware For_i loop (timing harness
    use only: amortizes host dispatch overhead across reps executions).
    """
    import concourse.mybir as mybir
    import concourse.tile as tile
    from concourse import bacc

    f32 = mybir.dt.float32
    f32r = mybir.dt.float32r
    DQ = HQ_ * D_

    nc = bacc.Bacc("TRN2", target_bir_lowering=False, debug=False,
                   num_devices=n_cores)
    bf16 = mybir.dt.bfloat16
    wo_dt = bf16 if version >= 3 else f32r
    y_dt = bf16 if version >= 3 else f32
    in_dt = bf16 if version >= 3 else f32r
    cos_dt = bf16 if version >= 4 else f32
    swp_dt = bf16 if version >= 4 else f32r
    io = {
        "xT": nc.dram_tensor("xT", [E_, S_], in_dt, kind="ExternalInput").ap(),
        "wqT": nc.dram_tensor("wqT", [E_, DQ], in_dt, kind="ExternalInput").ap(),
        "wkT": nc.dram_tensor("wkT", [E_, D_], in_dt, kind="ExternalInput").ap(),
        "wvT": nc.dram_tensor("wvT", [E_, D_], in_dt, kind="ExternalInput").ap(),
        "woT": nc.dram_tensor("woT", [DQ, E_], wo_dt, kind="ExternalInput").ap(),
        "cosd": nc.dram_tensor("cosd", [D_, S_], cos_dt, kind="ExternalInput").ap(),
        "sind": nc.dram_tensor("sind", [D_, S_], f32, kind="ExternalInput").ap(),
        "swp": nc.dram_tensor("swp", [D_, D_], swp_dt, kind="ExternalInput").ap(),
        "yT": nc.dram_tensor("yT", [E_, S_], y_dt, kind="ExternalOutput").ap(),
    }
    if version >= 4:
        io["tri"] = nc.dram_tensor("tri", [128, 128], bf16,
                                   kind="ExternalInput").ap()
    def emit(tc, mid_cb=None):
        if version == 4:
            emit_attention_v4(tc, io, S_=S_, E_=E_, HQ_=HQ_, D_=D_, CH=CH,
                              mid_cb=mid_cb)
        elif version == 3:
            emit_attention_v3(tc, io, S_=S_, E_=E_, HQ_=HQ_, D_=D_, CH=CH,
                              merge_strip=merge_strip)
        elif version == 2:
            emit_attention_v2(tc, io, S_=S_, E_=E_, HQ_=HQ_, D_=D_, CH=CH,
                              phases=phases)
        else:
            emit_attention(tc, io, S_=S_, E_=E_, HQ_=HQ_, D_=D_, CH=CH)

    with tile.TileContext(nc) as tc:
        if reps == 1:
            emit(tc)
        elif reps < 0:
            # straight-line unroll (-reps bodies, no For_i back-edge);
            # used to isolate the hardware loop's per-iteration sync cost
            for _ in range(-reps):
                emit(tc)
        else:
            # 2x unroll inside the hardware loop: the interior body-body
            # boundary has no branch / sem-reset, so the Tile scheduler
            # pipelines across it; only every other boundary pays the
            # back-edge (IRAM refetch + DMA restart) cost.
            unroll = 1
            if version >= 4:
                for u in (4, 2):
                    if reps % u == 0:
                        unroll = u
                        break
            with tc.For_i(0, reps // unroll, 1,
                          hint_engines=(mybir.EngineType.PE,
                                        mybir.EngineType.DVE,
                                        mybir.EngineType.Activation,
                                        mybir.EngineType.Pool,
                                        mybir.EngineType.SP),
                          staggered_reset=True):
                for ui in range(unroll):
                    emit(tc)
    nc.compile()
    return nc


# ---------------------------------------------------------------------------
# Host-side sharding / gather
# ---------------------------------------------------------------------------

def round_fp32r(a):
    """Round fp32 array to fp32r (11-bit mantissa, RNE), keeping fp32 layout."""
    b = np.ascontiguousarray(a, dtype=np.float32).view(np.uint32)
    b = b + np.uint32(0x7FF) + ((b >> np.uint32(12)) & np.uint32(1))
    b &= np.uint32(0xFFFFF000)
    return b.view(np.float32)


def shard_inputs(x, wq_w, wk_w, wv_w, wo_w, freqs_cos, freqs_sin,
                 S_=S, E_=E, HQ_=HQ, D_=D, groups=GROUPS, n_batch=B,
                 version=4):
    """Build per-core input maps (list ordered core 0..n-1, c = b*groups + g)."""
    import ml_dtypes
    bf = ml_dtypes.bfloat16
    x = np.asarray(x, dtype=np.float32)
    wq_w = np.asarray(wq_w, dtype=np.float32)
    wk_w = np.asarray(wk_w, dtype=np.float32)
    wv_w = np.asarray(wv_w, dtype=np.float32)
    wo_w = np.asarray(wo_w, dtype=np.float32)
    fc = np.asarray(freqs_cos, dtype=np.float32)  # [S, D/2]
    fs = np.asarray(freqs_sin, dtype=np.float32)

    cosd = np.repeat(fc.T, 2, axis=0)  # [D, S]
    sind = np.repeat(fs.T, 2, axis=0)
    sign = np.where(np.arange(D_) % 2 == 0, -1.0, 1.0).astype(np.float32)
    sind = sind * sign[:, None]
    if version >= 4:
        cosd = cosd.astype(bf)
    swp = np.zeros((D_, D_), dtype=np.float32)
    idx = np.arange(0, D_, 2)
    swp[idx, idx + 1] = 1.0
    swp[idx + 1, idx] = 1.0
    if version >= 4:
        swp = swp.astype(bf)  # exact 0/1 values
    r = np.arange(128)
    tri = (r[None, :] >= r[:, None]).astype(bf)  # keep col >= row

    NE = E_ // 128
    CHS = 512
    NSQ = S_ // CHS

    def pmajor_w(wT):  # [E, n] -> [128, NE, n]
        return np.ascontiguousarray(
            wT.reshape(NE, 128, wT.shape[1]).transpose(1, 0, 2)).astype(bf)

    in_maps = []
    for c in range(n_batch * groups):
        b, g = divmod(c, groups)
        heads = [g + groups * mm for mm in range(HQ_)]
        wq_rows = np.concatenate([wq_w[h * D_:(h + 1) * D_] for h in heads])  # [DQ, E]
        wo_cols = np.concatenate([wo_w[:, h * D_:(h + 1) * D_] for h in heads],
                                 axis=1)  # [E, DQ]
        woT = np.ascontiguousarray(wo_cols.T).astype(bf)
        if version >= 4:
            im = {
                "xT": np.ascontiguousarray(x[b].T).astype(bf),
                "wqT": np.ascontiguousarray(wq_rows.T).astype(bf),
                "wkT": np.ascontiguousarray(
                    wk_w[g * D_:(g + 1) * D_].T).astype(bf),
                "wvT": np.ascontiguousarray(
                    wv_w[g * D_:(g + 1) * D_].T).astype(bf),
                "woT": woT,
                "cosd": cosd,
                "sind": sind,
                "swp": swp,
                "tri": tri,
            }
        else:
            im = {
                "xT": np.ascontiguousarray(x[b].T).astype(bf),
                "wqT": np.ascontiguousarray(wq_rows.T).astype(bf),
                "wkT": np.ascontiguousarray(
                    wk_w[g * D_:(g + 1) * D_].T).astype(bf),
                "wvT": np.ascontiguousarray(
                    wv_w[g * D_:(g + 1) * D_].T).astype(bf),
                "woT": woT,
                "cosd": cosd,
                "sind": sind,
                "swp": swp,
            }
        in_maps.append(im)
    return in_maps


def gather_output(results, S_=S, E_=E, groups=GROUPS, n_batch=B):
    """results: list of per-core dicts with 'yT' [E, S] -> y [B, S, E]."""
    out = np.zeros((n_batch, S_, E_), dtype=np.float32)
    for c, res in enumerate(results):
        b = c // groups
        out[b] += res["yT"].T.astype(np.float32)
    return out


# ---------------------------------------------------------------------------
# Per-core numpy reference (for validation in tests)
# ---------------------------------------------------------------------------

def percore_ref(xT, wqT, wkT, wvT, woT, fc, fs, HQ_=HQ, D_=D):
    x = xT.T.astype(np.float64)  # [S, E]
    S_ = x.shape[0]
    q = x @ wqT.astype(np.float64)  # [S, DQ]
    k = x @ wkT.astype(np.float64)  # [S, D]
    v = x @ wvT.astype(np.float64)

    def rope(t):  # [S, n, D]
        tr = t.reshape(*t.shape[:-1], -1, 2)
        xr, xi = tr[..., 0], tr[..., 1]
        c = fc[:, None, :]
        s = fs[:, None, :]
        orr = xr * c - xi * s
        oi = xr * s + xi * c
        return np.stack([orr, oi], axis=-1).reshape(t.shape)

    q = rope(q.reshape(S_, HQ_, D_))
    k = rope(k.reshape(S_, 1, D_))[:, 0]
    out = np.zeros((S_, HQ_ * D_))
    causal = np.tril(np.ones((S_, S_), dtype=bool))
    for m in range(HQ_):
        sc = q[:, m] @ k.T / math.sqrt(D_)
        sc = np.where(causal, sc, -np.inf)
        sc = sc - sc.max(axis=-1, keepdims=True)
        p = np.exp(sc)
        p /= p.sum(axis=-1, keepdims=True)
        out[:, m * D_:(m + 1) * D_] = p @ v
    y = out @ woT.astype(np.float64)  # [S, E]
    return np.ascontiguousarray(y.T.astype(np.float32))  # yT


# ---------------------------------------------------------------------------
# Entry point
# ---------------------------------------------------------------------------

_NC_CACHE = {}


def _get_nc():
    if "nc" not in _NC_CACHE:
        _NC_CACHE["nc"] = build_nc()
    return _NC_CACHE["nc"]


def kernel(x, wq_w, wk_w, wv_w, wo_w, freqs_cos, freqs_sin, start_pos=0,
           **_ignored):
    from concourse.bass_utils import run_bass_kernel_spmd

    nc = _get_nc()
    in_maps = shard_inputs(x, wq_w, wk_w, wv_w, wo_w, freqs_cos, freqs_sin)
    res = run_bass_kernel_spmd(nc, in_maps, list(range(N_CORES)))
    return gather_output(res.results)

